# revision 10
# baseline (speedup 1.0000x reference)
"""GQA attention kernel for Trainium2, 8 NeuronCores — fp16, software-pipelined.

Problem: B=2, T=2048, D=1024, 16 Q heads / 4 KV heads, head_dim=64, RoPE,
causal softmax, out-projection.

Sharding: 8 cores = 2 (batch) x 4 (KV group). Core c handles batch c//4 and
KV group g=c%4 (query heads 4g..4g+3). wq/wk/wv column-sharded, wo
row-sharded; the 4 partial outputs per batch are summed on the host.

All matmul operands are fp16 (PSUM accumulates fp32); rel err ~6e-4 vs the
fp64 reference. Every stationary operand is padded to 128 columns so Fast
Weight Load triggers and LDWEIGHTS hides under the previous matmul.

Layout is transposed (head_dim on partitions): xT (D,T), qT (256,T),
kT (64,T dup'd to 128), scoresT[j,i] = k_j.q_i. Softmax computes
exp(s/8 - 4): the bias keeps unnormalized weights in fp16 range and cancels
exactly through 1/L. Causality: the diagonal 512x512 region of each query
chunk is computed TRIMMED — the four 128-key blocks only cover query
columns [128r:512), and all four residual triangles are the SAME [128,128]
0/1 matrix, applied multiplicatively to the fp16 `at` tile after exp
(split over DVE and GpSimd; GpSimd cannot touch PSUM). L rides the PV
matmul as a ones-column of v; 1/L is computed lane-parallel by packing the
L row [1,512] into [128,4] via SBUF-SBUF DMA, then broadcast back over 64
partitions with a ones-stationary matmul.

Scheduling: engines are strict-FIFO, so emission order is the schedule.
The main loop runs query-chunk rounds (ci-outer, heads inner) and weaves
"filler" PE work — next round's K/V/Q projection chunks, previous round's
output-projection tiles — between attention groups, so the PE queue never
drains while ACT grinds exp (ACT is the attention-phase pacer at
(N+352)/1.2 ns). A drained PE triggers the HAM clock gate (PE drops
2.4 -> 1.2 GHz), which is what made previous versions 2x slow. Each head's
L-chain/normalize is deferred into the next head's stretch so its DMA
round-trips never block the ACT/DVE queues.
"""

import numpy as np
import sys
from collections import deque

sys.path.insert(0, "/opt/trn_rl_repo")

from concourse import bass, bacc, mybir, tile  # noqa: E402
from concourse.bass_utils import run_bass_kernel_spmd  # noqa: E402

F32 = mybir.dt.float32
F16 = mybir.dt.float16
EXP = mybir.ActivationFunctionType.Exp

B, T, D = 2, 2048, 1024
HD = 64                      # head dim
NQH = 4                      # query heads per core
QCOLS = NQH * HD             # 256
KC = D // 128                # 8 contraction chunks
NT = T // 128                # 16 row tiles
NC4 = T // 512               # 4 512-wide column chunks
N_CORES = 8

# trimmed diagonal geometry: block r covers query cols [TRIM_OFF[r]:512),
# packed into diag-a (r0,r1,r3) + diag-b (r2) exp groups.
TRIM_OFF = [0, 128, 256, 384]

_cache = {}


def build_nc():
    nc = bacc.Bacc("TRN2", target_bir_lowering=False, debug=False)

    xT_d = nc.declare_dram_parameter("xT", [D, T], F16, isOutput=False)
    wq_d = nc.declare_dram_parameter("wq", [D, QCOLS], F16, isOutput=False)
    wk_d = nc.declare_dram_parameter("wk", [D, 128], F16, isOutput=False)
    wv_d = nc.declare_dram_parameter("wv", [D, 128], F16, isOutput=False)
    wo_d = nc.declare_dram_parameter("wo", [QCOLS, D], F16, isOutput=False)
    cos_d = nc.declare_dram_parameter("cosf", [128, T], F16, isOutput=False)
    sin_d = nc.declare_dram_parameter("sinf", [128, T], F16, isOutput=False)
    tri_d = nc.declare_dram_parameter("tri", [128, 128], F16, isOutput=False)
    idn_d = nc.declare_dram_parameter("iden", [64, 64], F16, isOutput=False)
    out_d = nc.declare_dram_parameter("out", [T, D], F16, isOutput=True)

    with tile.TileContext(nc) as tc:
        with (
            tc.tile_pool(name="sb", bufs=1) as sb,
            tc.tile_pool(name="sbx", bufs=1) as sbx,
            tc.tile_pool(name="rope", bufs=2) as rope_pool,
            tc.tile_pool(name="at", bufs=3) as at_pool,
            tc.tile_pool(name="aox", bufs=2) as aox,
            tc.tile_pool(name="outp", bufs=3) as outp,
            tc.tile_pool(name="wrk", bufs=1, space="PSUM") as wrk,
            tc.tile_pool(name="scp", bufs=2, space="PSUM") as scp,
            tc.tile_pool(name="pvp", bufs=2, space="PSUM") as pvp,
        ):
            wq = sb.tile([128, KC, QCOLS], F16, tag="wq")
            wk = sb.tile([128, KC, 128], F16, tag="wk")
            wv = sb.tile([128, KC, 128], F16, tag="wv")
            wo = sb.tile([128, 2, D], F16, tag="wo")
            cosf = sb.tile([128, T], F16, tag="cosf")
            sinf = sb.tile([128, T], F16, tag="sinf")
            tri = sb.tile([128, 128], F16, tag="tri")
            iden = sb.tile([64, 64], F16, tag="iden")
            onesr = sb.tile([1, 128], F16, tag="onesr")
            bias = sb.tile([128, 1], F32, tag="bias")
            # per-512-chunk tiles (chunk-grain independence for the pipeline)
            qTc = [[sb.tile([128, 512], F16, tag=f"qT{hp}_{ci}",
                            name=f"qT{hp}_{ci}") for ci in range(NC4)]
                   for hp in range(2)]
            kTc = [sb.tile([128, 512], F16, tag=f"kT{ci}", name=f"kT{ci}")
                   for ci in range(NC4)]
            vTc = [sb.tile([64, 512], F16, tag=f"vT{ci}", name=f"vT{ci}")
                   for ci in range(NC4)]
            vc = [sb.tile([128, 4, 128], F16, tag=f"v{ci}", name=f"v{ci}")
                  for ci in range(NC4)]
            ao = [[sb.tile([128, 512], F16, tag=f"ao{hp}_{ci}",
                           name=f"ao{hp}_{ci}") for ci in range(NC4)]
                  for hp in range(2)]
            xT = sbx.tile([128, KC, T], F16, tag="xT")

            nc.gpsimd.memset(onesr[:], 1.0)
            nc.gpsimd.memset(bias[:], -4.0)
            for ci in range(NC4):
                nc.gpsimd.memset(vc[ci][:], 0.0)
                nc.gpsimd.memset(vc[ci][:, :, HD:HD + 1], 1.0)

            for k in range(KC):
                nc.sync.dma_start(xT[:, k, :], xT_d[k * 128:(k + 1) * 128, :])
            for k in range(KC):
                nc.sync.dma_start(wk[:, k, :], wk_d[k * 128:(k + 1) * 128, :])
                nc.sync.dma_start(wv[:, k, :], wv_d[k * 128:(k + 1) * 128, :])
                nc.sync.dma_start(wq[:, k, :], wq_d[k * 128:(k + 1) * 128, :])
            nc.sync.dma_start(cosf[:], cos_d[:])
            nc.sync.dma_start(sinf[:], sin_d[:])
            nc.sync.dma_start(tri[:], tri_d[:])
            nc.sync.dma_start(iden[:], idn_d[:])
            for c in range(2):
                nc.sync.dma_start(wo[:, c, :], wo_d[c * 128:(c + 1) * 128, :])

            # ---------- emission helpers ----------
            def rope_chunk(q_ap, nrows, cs):
                """in-place RoPE on a [*, 512] chunk tile (cs indexes cos/sin)."""
                rot = rope_pool.tile([128, 512], F16, tag="rot", bufs=4)
                for blk in range(nrows // 64):
                    r0 = blk * 64
                    nc.gpsimd.dma_start(rot[r0:r0 + 32, :],
                                        q_ap[r0 + 32:r0 + 64, :])
                    nc.gpsimd.dma_start(rot[r0 + 32:r0 + 64, :],
                                        q_ap[r0:r0 + 32, :])
                nc.vector.tensor_mul(q_ap[0:nrows, :], q_ap[0:nrows, :],
                                     cosf[0:nrows, cs])
                nc.vector.tensor_mul(rot[0:nrows, :], rot[0:nrows, :],
                                     sinf[0:nrows, cs])
                nc.vector.tensor_add(q_ap[0:nrows, :], q_ap[0:nrows, :],
                                     rot[0:nrows, :])

            def proj_k(ci):
                cs = slice(ci * 512, (ci + 1) * 512)
                p = wrk.tile([128, 2, 512], F32, tag="wk_")
                for k in range(KC):
                    nc.tensor.matmul(p[:, 0, :], wk[:, k, :], xT[:, k, cs],
                                     start=(k == 0), stop=(k == KC - 1))
                nc.vector.tensor_copy(kTc[ci][0:64, :], p[0:64, 0, :])
                rope_chunk(kTc[ci], 64, cs)
                nc.sync.dma_start(kTc[ci][64:128, :], kTc[ci][0:64, :])

            def proj_v(ci):
                cs = slice(ci * 512, (ci + 1) * 512)
                p = wrk.tile([128, 2, 512], F32, tag="wk_")
                for k in range(KC):
                    nc.tensor.matmul(p[:, 0, :], wv[:, k, :], xT[:, k, cs],
                                     start=(k == 0), stop=(k == KC - 1))
                nc.vector.tensor_copy(vTc[ci][0:64, :], p[0:64, 0, :])

            def trans_v(ci):
                vtr = wrk.tile([128, 4, 64], F16, tag="wk_")
                for tt in range(4):
                    nc.tensor.transpose(vtr[:, tt, :],
                                        vTc[ci][:, tt * 128:(tt + 1) * 128],
                                        iden[:])
                    nc.vector.tensor_copy(vc[ci][:, tt, 0:HD], vtr[:, tt, :])

            def proj_q(hp, ci):
                cs = slice(ci * 512, (ci + 1) * 512)
                p = wrk.tile([128, 2, 512], F32, tag="wk_")
                for k in range(KC):
                    nc.tensor.matmul(
                        p[:, 0, :], wq[:, k, hp * 128:(hp + 1) * 128],
                        xT[:, k, cs], start=(k == 0), stop=(k == KC - 1))
                nc.vector.tensor_copy(qTc[hp][ci][:], p[:, 0, :])
                rope_chunk(qTc[hp][ci], 128, cs)

            def po_tile(t):
                ci, tt = divmod(t, 4)
                toff = slice(tt * 128, (tt + 1) * 128)
                po = wrk.tile([128, 2, 512], F32, tag="wk_")
                for nh in range(2):
                    ns = slice(nh * 512, (nh + 1) * 512)
                    for cc in range(2):
                        nc.tensor.matmul(
                            po[:, nh, :], ao[cc][ci][:, toff], wo[:, cc, ns],
                            start=(cc == 0), stop=(cc == 1))
                ot = outp.tile([128, D], F16, tag="ot")
                nc.vector.tensor_copy(ot[:, 0:512], po[:, 0, :])
                nc.vector.tensor_copy(ot[:, 512:1024], po[:, 1, :])
                nc.sync.dma_start(out_d[t * 128:(t + 1) * 128, :], ot[:])

            def kchunk(tj):
                """[64 or 128, 128] stationary slice for key tile tj."""
                return kTc[tj // 4], (tj % 4) * 128

            def head_groups(ci, h):
                """Build (emit_sc, emit_pv) closure pairs for head h, chunk ci,
                plus the deferred L-chain closure. pv emission is skewed one
                group behind sc/exp by the caller so the PE never head-of-line
                blocks on the last exp of a head."""
                hp, hr = divmod(h, 2)
                qrow = slice(hr * 64, hr * 64 + 64)
                qt = qTc[hp][ci]
                pv = pvp.tile([128, 512], F32, tag="pv", name=f"pv{ci}_{h}")
                n_off = ci * 4
                groups = []
                for tg in range(n_off // 2):
                    cell = {}
                    def em_sc(tg=tg, cell=cell):
                        scg = scp.tile([128, 2, 512], F32, tag="sc")
                        for j in range(2):
                            tj = tg * 2 + j
                            kt, ko = kchunk(tj)
                            nc.tensor.matmul(
                                scg[:, j, :], kt[qrow, ko:ko + 128],
                                qt[qrow, :], start=True, stop=True)
                        atg = at_pool.tile([128, 2, 512], F16, tag="at")
                        nc.scalar.activation(atg[:], scg[:], EXP,
                                             scale=0.125, bias=bias[:])
                        cell["at"] = atg
                    def em_pv(tg=tg, cell=cell):
                        atg = cell["at"]
                        for j in range(2):
                            tj = tg * 2 + j
                            nc.tensor.matmul(pv[:], vc[tj // 4][:, tj % 4, :],
                                             atg[:, j, :],
                                             start=(tj == 0), stop=False)
                    groups.append((em_sc, em_pv))
                # diagonal item (trimmed): r0@[0:512], r1@[512:896], r3@[896:1024]
                cell = {}
                def em_sc_d(cell=cell):
                    kt, _ = kchunk(n_off)
                    sca = scp.tile([128, 2, 512], F32, tag="sc")
                    nc.tensor.matmul(sca[:, 0, :], kt[qrow, 0:128],
                                     qt[qrow, :], start=True, stop=True)
                    nc.tensor.matmul(sca[:, 1, 0:384], kt[qrow, 128:256],
                                     qt[qrow, 128:512], start=True, stop=True)
                    nc.tensor.matmul(sca[:, 1, 384:512], kt[qrow, 384:512],
                                     qt[qrow, 384:512], start=True, stop=True)
                    ata = at_pool.tile([128, 2, 512], F16, tag="at")
                    nc.scalar.activation(ata[:], sca[:], EXP,
                                         scale=0.125, bias=bias[:])
                    scb = scp.tile([128, 2, 512], F32, tag="sc")
                    nc.tensor.matmul(scb[:, 0, 0:256], kt[qrow, 256:384],
                                     qt[qrow, 256:512], start=True, stop=True)
                    atb = at_pool.tile([128, 256], F16, tag="at")
                    nc.scalar.activation(atb[:], scb[:, 0, 0:256], EXP,
                                         scale=0.125, bias=bias[:])
                    nc.vector.tensor_mul(ata[:, 0, 0:128], ata[:, 0, 0:128],
                                         tri[:])
                    nc.gpsimd.tensor_mul(ata[:, 1, 0:128], ata[:, 1, 0:128],
                                         tri[:])
                    nc.gpsimd.tensor_mul(ata[:, 1, 384:512],
                                         ata[:, 1, 384:512], tri[:])
                    nc.vector.tensor_mul(atb[:, 0:128], atb[:, 0:128], tri[:])
                    cell["a"], cell["b"] = ata, atb
                def em_pv_d(cell=cell):
                    ata, atb = cell["a"], cell["b"]
                    vd = vc[ci]
                    nc.tensor.matmul(pv[:, 0:512], vd[:, 0, :], ata[:, 0, :],
                                     start=(n_off == 0), stop=False)
                    nc.tensor.matmul(pv[:, 128:512], vd[:, 1, :],
                                     ata[:, 1, 0:384], start=False, stop=False)
                    nc.tensor.matmul(pv[:, 384:512], vd[:, 3, :],
                                     ata[:, 1, 384:512], start=False,
                                     stop=False)
                    nc.tensor.matmul(pv[:, 256:512], vd[:, 2, :], atb[:],
                                     start=False, stop=True)
                groups.append((em_sc_d, em_pv_d))

                def lchain():
                    lrow = aox.tile([1, 512], F32, tag="lrow")
                    nc.vector.tensor_copy(lrow[:], pv[64:65, :])
                    pkl = aox.tile([128, 4], F32, tag="pkl")
                    nc.gpsimd.dma_start(pkl[:], lrow[:])
                    rcl = aox.tile([128, 4], F16, tag="rcl")
                    with nc.allow_low_precision(reason="fp16 linv"):
                        nc.vector.reciprocal(rcl[:], pkl[:])
                    linv = aox.tile([1, 512], F16, tag="linv")
                    nc.gpsimd.dma_start(linv[:], rcl[:])
                    lb = scp.tile([128, 512], F32, tag="sc")
                    nc.tensor.matmul(lb[:], onesr[:], linv[:],
                                     start=True, stop=True)
                    lbs = aox.tile([64, 512], F16, tag="lbs")
                    nc.vector.tensor_copy(lbs[:], lb[0:64, :])
                    if hr == 0:
                        dst = ao[hp][ci][0:64, :]
                    else:
                        dst = aox.tile([64, 512], F16, tag="aotmp")
                    nc.vector.tensor_mul(dst, pv[0:64, :], lbs[:])
                    if hr == 1:
                        nc.sync.dma_start(ao[hp][ci][64:128, :], dst)
                return groups, lchain

            # ---------- schedule ----------
            # prologue: chunk-0 projections
            proj_k(0)
            proj_v(0)
            trans_v(0)
            proj_q(0, 0)
            proj_q(1, 0)

            projf = deque()   # (due_chunk, closure): before round due_chunk
            pof = deque()     # out-projection fillers, consumed in late rounds

            def pop_filler(allow_po):
                if projf:
                    projf.popleft()[1]()
                    return True
                if pof and allow_po:
                    pof.popleft()()
                    return True
                return False

            for ci in range(NC4):
                while projf and projf[0][0] <= ci:   # safety drain
                    projf.popleft()[1]()
                if ci + 1 < NC4:
                    c = ci + 1
                    projf.append((c, lambda c=c: proj_k(c)))
                    projf.append((c, lambda c=c: proj_v(c)))
                    projf.append((c, lambda c=c: trans_v(c)))
                    projf.append((c, lambda c=c: proj_q(0, c)))
                    projf.append((c, lambda c=c: proj_q(1, c)))
                # build all head group-items for this round
                items = []          # (emit_sc, emit_pv, head)
                lchains = {}
                for h in range(NQH):
                    groups, lc = head_groups(ci, h)
                    lchains[h] = lc
                    for (es, ep) in groups:
                        items.append((es, ep, h))
                allow_po = ci >= 2
                due = deque()       # closures to fire one item later
                for i, (es, ep, h) in enumerate(items):
                    es()
                    if i > 0:
                        items[i - 1][1]()            # skewed pv of prev item
                        if items[i - 1][2] != h:     # head boundary crossed:
                            due.append(lchains[items[i - 1][2]])
                    if due:
                        due.popleft()()
                    elif i % 2 == 1:
                        pop_filler(allow_po)
                items[-1][1]()
                lchains[items[-1][2]]()
                for tt in range(4):
                    pof.append(lambda t=ci * 4 + tt: po_tile(t))
            while projf or pof:
                pop_filler(True)

    nc.compile()
    return nc


def make_in_maps(x, freqs_cos, freqs_sin, wq, wk, wv, wo):
    """Host-side sharding + layout prep. Returns per-core input dicts."""
    f16 = np.float16
    x = np.asarray(x, np.float32)
    fc = np.asarray(freqs_cos, np.float32)
    fs = np.asarray(freqs_sin, np.float32)
    wq = np.asarray(wq, np.float32)
    wk = np.asarray(wk, np.float32)
    wv = np.asarray(wv, np.float32)
    wo = np.asarray(wo, np.float32)

    perm = np.concatenate([np.arange(0, HD, 2), np.arange(1, HD, 2)])
    cosT = np.ascontiguousarray(fc.T)            # (32, T)
    sinT = np.ascontiguousarray(fs.T)
    cosf = np.concatenate([cosT] * 4, axis=0).astype(f16)    # (128, T)
    sinf = np.concatenate([-sinT, sinT, -sinT, sinT], axis=0).astype(f16)

    jj = np.arange(128)[:, None]
    cc_ = np.arange(128)[None, :]
    tri = (jj <= cc_).astype(f16)                # [key j, query c]
    iden = np.eye(64, dtype=f16)

    def pad128(w):  # (D, 64) -> (D, 128)
        z = np.zeros((D, 128), f16)
        z[:, 0:HD] = w
        return z

    in_maps = []
    for c in range(N_CORES):
        b, g = divmod(c, 4)
        wq_c = wq[:, g * QCOLS:(g + 1) * QCOLS]
        wq_c = np.ascontiguousarray(
            wq_c.reshape(D, NQH, HD)[:, :, perm].reshape(D, QCOLS)).astype(f16)
        wk_c = pad128(wk[:, g * HD:(g + 1) * HD][:, perm].astype(f16))
        wv_c = pad128(wv[:, g * HD:(g + 1) * HD].astype(f16))
        wo_c = np.ascontiguousarray(wo[g * QCOLS:(g + 1) * QCOLS, :]).astype(f16)
        xT_c = np.ascontiguousarray(x[b].T).astype(f16)
        in_maps.append({
            "xT": xT_c, "wq": wq_c, "wk": wk_c, "wv": wv_c, "wo": wo_c,
            "cosf": cosf, "sinf": sinf, "tri": tri, "iden": iden,
        })
    return in_maps


def run_on_cores(in_maps, trace=False, **kwargs):
    if "nc" not in _cache:
        _cache["nc"] = build_nc()
    return run_bass_kernel_spmd(
        _cache["nc"], in_maps, core_ids=list(range(N_CORES)), trace=trace,
        **kwargs)


def kernel(x, freqs_cos, freqs_sin, wq, wk, wv, wo):
    in_maps = make_in_maps(x, freqs_cos, freqs_sin, wq, wk, wv, wo)
    res = run_on_cores(in_maps)
    outs = [np.asarray(res.results[c]["out"], np.float32)
            for c in range(N_CORES)]
    full = np.empty((B, T, D), np.float32)
    for b in range(B):
        full[b] = outs[4 * b] + outs[4 * b + 1] + outs[4 * b + 2] + outs[4 * b + 3]
    return full


# revision 11
# speedup vs baseline: 1.4073x; 1.4073x over previous
"""GQA attention kernel for Trainium2, 8 NeuronCores — fp16, software-pipelined.

Problem: B=2, T=2048, D=1024, 16 Q heads / 4 KV heads, head_dim=64, RoPE,
causal softmax, out-projection.

Sharding: 8 cores = 2 (batch) x 4 (KV group). Core c handles batch c//4 and
KV group g=c%4 (query heads 4g..4g+3). wq/wk/wv column-sharded, wo
row-sharded; the 4 partial outputs per batch are summed on the host.

All matmul operands are fp16 (PSUM accumulates fp32); rel err ~6e-4 vs the
fp64 reference. Every stationary operand is padded to 128 columns so Fast
Weight Load triggers and LDWEIGHTS hides under the previous matmul.

Layout is transposed (head_dim on partitions): xT (D,T), qT (256,T),
kT (64,T dup'd to 128), scoresT[j,i] = k_j.q_i. Softmax computes
exp(s/8 - 4): the bias keeps unnormalized weights in fp16 range and cancels
exactly through 1/L. Causality: the diagonal 512x512 region of each query
chunk is computed TRIMMED — the four 128-key blocks only cover query
columns [128r:512), and all four residual triangles are the SAME [128,128]
0/1 matrix, applied multiplicatively to the fp16 `at` tile after exp
(split over DVE and GpSimd; GpSimd cannot touch PSUM). L rides the PV
matmul as a ones-column of v; 1/L is computed lane-parallel by packing the
L row [1,512] into [128,4] via SBUF-SBUF DMA, then broadcast back over 64
partitions with a ones-stationary matmul.

Scheduling: engines are strict-FIFO, so emission order is the schedule.
The main loop runs query-chunk rounds (ci-outer, heads inner) and weaves
"filler" PE work — next round's K/V/Q projection chunks, previous round's
output-projection tiles — between attention groups, so the PE queue never
drains while ACT grinds exp (ACT is the attention-phase pacer at
(N+352)/1.2 ns). A drained PE triggers the HAM clock gate (PE drops
2.4 -> 1.2 GHz), which is what made previous versions 2x slow. Each head's
L-chain/normalize is deferred into the next head's stretch so its DMA
round-trips never block the ACT/DVE queues.
"""

import numpy as np
import sys
from collections import deque

sys.path.insert(0, "/opt/trn_rl_repo")

from concourse import bass, bacc, mybir, tile  # noqa: E402
from concourse.bass_utils import run_bass_kernel_spmd  # noqa: E402

F32 = mybir.dt.float32
F16 = mybir.dt.float16
EXP = mybir.ActivationFunctionType.Exp

B, T, D = 2, 2048, 1024
HD = 64                      # head dim
NQH = 4                      # query heads per core
QCOLS = NQH * HD             # 256
KC = D // 128                # 8 contraction chunks
NT = T // 128                # 16 row tiles
NC4 = T // 512               # 4 512-wide column chunks
N_CORES = 8

# trimmed diagonal geometry: block r covers query cols [TRIM_OFF[r]:512),
# packed into diag-a (r0,r1,r3) + diag-b (r2) exp groups.
TRIM_OFF = [0, 128, 256, 384]

_cache = {}


def build_nc():
    nc = bacc.Bacc("TRN2", target_bir_lowering=False, debug=False)

    xT_d = nc.declare_dram_parameter("xT", [D, T], F16, isOutput=False)
    wq_d = nc.declare_dram_parameter("wq", [D, QCOLS], F16, isOutput=False)
    wk_d = nc.declare_dram_parameter("wk", [D, 128], F16, isOutput=False)
    wv_d = nc.declare_dram_parameter("wv", [D, 128], F16, isOutput=False)
    wo_d = nc.declare_dram_parameter("wo", [QCOLS, D], F16, isOutput=False)
    cos_d = nc.declare_dram_parameter("cosf", [128, T], F16, isOutput=False)
    sin_d = nc.declare_dram_parameter("sinf", [128, T], F16, isOutput=False)
    tri_d = nc.declare_dram_parameter("tri", [128, 128], F16, isOutput=False)
    idn_d = nc.declare_dram_parameter("iden", [64, 64], F16, isOutput=False)
    out_d = nc.declare_dram_parameter("out", [T, D], F16, isOutput=True)

    with tile.TileContext(nc) as tc:
        with (
            tc.tile_pool(name="sb", bufs=1) as sb,
            tc.tile_pool(name="sbx", bufs=1) as sbx,
            tc.tile_pool(name="rope", bufs=2) as rope_pool,
            tc.tile_pool(name="at", bufs=3) as at_pool,
            tc.tile_pool(name="aox", bufs=2) as aox,
            tc.tile_pool(name="outp", bufs=3) as outp,
            tc.tile_pool(name="wrk", bufs=1, space="PSUM") as wrk,
            tc.tile_pool(name="scp", bufs=2, space="PSUM") as scp,
            tc.tile_pool(name="pvp", bufs=2, space="PSUM") as pvp,
        ):
            wq = sb.tile([128, KC, QCOLS], F16, tag="wq")
            wk = sb.tile([128, KC, 128], F16, tag="wk")
            wv = sb.tile([128, KC, 128], F16, tag="wv")
            wo = sb.tile([128, 2, D], F16, tag="wo")
            cosf = sb.tile([128, T], F16, tag="cosf")
            sinf = sb.tile([128, T], F16, tag="sinf")
            tri = sb.tile([128, 128], F16, tag="tri")
            iden = sb.tile([64, 64], F16, tag="iden")
            onesr = sb.tile([1, 128], F16, tag="onesr")
            bias = sb.tile([128, 1], F32, tag="bias")
            # per-512-chunk tiles (chunk-grain independence for the pipeline)
            qTc = [[sb.tile([128, 512], F16, tag=f"qT{hp}_{ci}",
                            name=f"qT{hp}_{ci}") for ci in range(NC4)]
                   for hp in range(2)]
            kTc = [sb.tile([128, 512], F16, tag=f"kT{ci}", name=f"kT{ci}")
                   for ci in range(NC4)]
            vTc = [sb.tile([64, 512], F16, tag=f"vT{ci}", name=f"vT{ci}")
                   for ci in range(NC4)]
            vc = [sb.tile([128, 4, 128], F16, tag=f"v{ci}", name=f"v{ci}")
                  for ci in range(NC4)]
            ao = [[sb.tile([128, 512], F16, tag=f"ao{hp}_{ci}",
                           name=f"ao{hp}_{ci}") for ci in range(NC4)]
                  for hp in range(2)]
            xT = sbx.tile([128, KC, T], F16, tag="xT")

            nc.gpsimd.memset(onesr[:], 1.0)
            nc.gpsimd.memset(bias[:], -4.0)
            for ci in range(NC4):
                nc.gpsimd.memset(vc[ci][:], 0.0)
                nc.gpsimd.memset(vc[ci][:, :, HD:HD + 1], 1.0)

            for k in range(KC):
                nc.sync.dma_start(xT[:, k, :], xT_d[k * 128:(k + 1) * 128, :])
            for k in range(KC):
                nc.sync.dma_start(wk[:, k, :], wk_d[k * 128:(k + 1) * 128, :])
                nc.sync.dma_start(wv[:, k, :], wv_d[k * 128:(k + 1) * 128, :])
                nc.sync.dma_start(wq[:, k, :], wq_d[k * 128:(k + 1) * 128, :])
            nc.sync.dma_start(cosf[:], cos_d[:])
            nc.sync.dma_start(sinf[:], sin_d[:])
            nc.sync.dma_start(tri[:], tri_d[:])
            nc.sync.dma_start(iden[:], idn_d[:])
            for c in range(2):
                nc.sync.dma_start(wo[:, c, :], wo_d[c * 128:(c + 1) * 128, :])

            # ---------- emission helpers ----------
            def rope_chunk(q_ap, nrows, cs):
                """in-place RoPE on a [*, 512] chunk tile (cs indexes cos/sin)."""
                rot = rope_pool.tile([128, 512], F16, tag="rot", bufs=4)
                for blk in range(nrows // 64):
                    r0 = blk * 64
                    nc.sync.dma_start(rot[r0:r0 + 32, :],
                                      q_ap[r0 + 32:r0 + 64, :])
                    nc.sync.dma_start(rot[r0 + 32:r0 + 64, :],
                                      q_ap[r0:r0 + 32, :])
                nc.vector.tensor_mul(q_ap[0:nrows, :], q_ap[0:nrows, :],
                                     cosf[0:nrows, cs])
                nc.vector.tensor_mul(rot[0:nrows, :], rot[0:nrows, :],
                                     sinf[0:nrows, cs])
                nc.vector.tensor_add(q_ap[0:nrows, :], q_ap[0:nrows, :],
                                     rot[0:nrows, :])

            def proj_k(ci):
                cs = slice(ci * 512, (ci + 1) * 512)
                p = wrk.tile([128, 2, 512], F32, tag="wk_")
                for k in range(KC):
                    nc.tensor.matmul(p[:, 0, :], wk[:, k, :], xT[:, k, cs],
                                     start=(k == 0), stop=(k == KC - 1))
                nc.vector.tensor_copy(kTc[ci][0:64, :], p[0:64, 0, :])
                rope_chunk(kTc[ci], 64, cs)
                nc.sync.dma_start(kTc[ci][64:128, :], kTc[ci][0:64, :])

            def proj_v(ci):
                cs = slice(ci * 512, (ci + 1) * 512)
                p = wrk.tile([128, 2, 512], F32, tag="wk_")
                for k in range(KC):
                    nc.tensor.matmul(p[:, 0, :], wv[:, k, :], xT[:, k, cs],
                                     start=(k == 0), stop=(k == KC - 1))
                nc.vector.tensor_copy(vTc[ci][0:64, :], p[0:64, 0, :])

            def trans_v(ci):
                vtr = wrk.tile([128, 4, 64], F16, tag="wk_")
                for tt in range(4):
                    nc.tensor.transpose(vtr[:, tt, :],
                                        vTc[ci][:, tt * 128:(tt + 1) * 128],
                                        iden[:])
                    nc.vector.tensor_copy(vc[ci][:, tt, 0:HD], vtr[:, tt, :])

            def proj_q(hp, ci):
                cs = slice(ci * 512, (ci + 1) * 512)
                p = wrk.tile([128, 2, 512], F32, tag="wk_")
                for k in range(KC):
                    nc.tensor.matmul(
                        p[:, 0, :], wq[:, k, hp * 128:(hp + 1) * 128],
                        xT[:, k, cs], start=(k == 0), stop=(k == KC - 1))
                nc.vector.tensor_copy(qTc[hp][ci][:], p[:, 0, :])
                rope_chunk(qTc[hp][ci], 128, cs)

            def po_tile(t):
                ci, tt = divmod(t, 4)
                toff = slice(tt * 128, (tt + 1) * 128)
                po = wrk.tile([128, 2, 512], F32, tag="wk_")
                for nh in range(2):
                    ns = slice(nh * 512, (nh + 1) * 512)
                    for cc in range(2):
                        nc.tensor.matmul(
                            po[:, nh, :], ao[cc][ci][:, toff], wo[:, cc, ns],
                            start=(cc == 0), stop=(cc == 1))
                ot = outp.tile([128, D], F16, tag="ot")
                nc.vector.tensor_copy(ot[:, 0:512], po[:, 0, :])
                nc.vector.tensor_copy(ot[:, 512:1024], po[:, 1, :])
                nc.sync.dma_start(out_d[t * 128:(t + 1) * 128, :], ot[:])

            def kchunk(tj):
                """[64 or 128, 128] stationary slice for key tile tj."""
                return kTc[tj // 4], (tj % 4) * 128

            def head_groups(ci, h):
                """Build (emit_sc, emit_pv) closure pairs for head h, chunk ci,
                plus the deferred L-chain closure. pv emission is skewed one
                group behind sc/exp by the caller so the PE never head-of-line
                blocks on the last exp of a head."""
                hp, hr = divmod(h, 2)
                qrow = slice(hr * 64, hr * 64 + 64)
                qt = qTc[hp][ci]
                pv = pvp.tile([128, 512], F32, tag="pv", name=f"pv{ci}_{h}")
                n_off = ci * 4
                groups = []
                for tg in range(n_off // 2):
                    cell = {}
                    def em_sc(tg=tg, cell=cell):
                        scg = scp.tile([128, 2, 512], F32, tag="sc")
                        for j in range(2):
                            tj = tg * 2 + j
                            kt, ko = kchunk(tj)
                            nc.tensor.matmul(
                                scg[:, j, :], kt[qrow, ko:ko + 128],
                                qt[qrow, :], start=True, stop=True)
                        atg = at_pool.tile([128, 2, 512], F16, tag="at")
                        nc.scalar.activation(atg[:], scg[:], EXP,
                                             scale=0.125, bias=bias[:])
                        cell["at"] = atg
                    def em_pv(tg=tg, cell=cell):
                        atg = cell["at"]
                        for j in range(2):
                            tj = tg * 2 + j
                            nc.tensor.matmul(pv[:], vc[tj // 4][:, tj % 4, :],
                                             atg[:, j, :],
                                             start=(tj == 0), stop=False)
                    groups.append((em_sc, em_pv))
                # diagonal item (trimmed): r0@[0:512], r1@[512:896], r3@[896:1024]
                cell = {}
                def em_sc_d(cell=cell):
                    kt, _ = kchunk(n_off)
                    sca = scp.tile([128, 2, 512], F32, tag="sc")
                    nc.tensor.matmul(sca[:, 0, :], kt[qrow, 0:128],
                                     qt[qrow, :], start=True, stop=True)
                    nc.tensor.matmul(sca[:, 1, 0:384], kt[qrow, 128:256],
                                     qt[qrow, 128:512], start=True, stop=True)
                    nc.tensor.matmul(sca[:, 1, 384:512], kt[qrow, 384:512],
                                     qt[qrow, 384:512], start=True, stop=True)
                    ata = at_pool.tile([128, 2, 512], F16, tag="at")
                    nc.scalar.activation(ata[:], sca[:], EXP,
                                         scale=0.125, bias=bias[:])
                    scb = scp.tile([128, 2, 512], F32, tag="sc")
                    nc.tensor.matmul(scb[:, 0, 0:256], kt[qrow, 256:384],
                                     qt[qrow, 256:512], start=True, stop=True)
                    atb = at_pool.tile([128, 256], F16, tag="at")
                    nc.scalar.activation(atb[:], scb[:, 0, 0:256], EXP,
                                         scale=0.125, bias=bias[:])
                    nc.vector.tensor_mul(ata[:, 0, 0:128], ata[:, 0, 0:128],
                                         tri[:])
                    nc.gpsimd.tensor_mul(ata[:, 1, 0:128], ata[:, 1, 0:128],
                                         tri[:])
                    nc.gpsimd.tensor_mul(ata[:, 1, 384:512],
                                         ata[:, 1, 384:512], tri[:])
                    nc.vector.tensor_mul(atb[:, 0:128], atb[:, 0:128], tri[:])
                    cell["a"], cell["b"] = ata, atb
                def em_pv_d(cell=cell):
                    ata, atb = cell["a"], cell["b"]
                    vd = vc[ci]
                    nc.tensor.matmul(pv[:, 0:512], vd[:, 0, :], ata[:, 0, :],
                                     start=(n_off == 0), stop=False)
                    nc.tensor.matmul(pv[:, 128:512], vd[:, 1, :],
                                     ata[:, 1, 0:384], start=False, stop=False)
                    nc.tensor.matmul(pv[:, 384:512], vd[:, 3, :],
                                     ata[:, 1, 384:512], start=False,
                                     stop=False)
                    nc.tensor.matmul(pv[:, 256:512], vd[:, 2, :], atb[:],
                                     start=False, stop=True)
                groups.append((em_sc_d, em_pv_d))

                def lchain1():
                    lrow = aox.tile([1, 512], F32, tag="lrow")
                    nc.vector.tensor_copy(lrow[:], pv[64:65, :])
                    pkl = aox.tile([128, 4], F32, tag="pkl")
                    nc.gpsimd.dma_start(pkl[:], lrow[:])
                    cellL["pkl"] = pkl

                def lchain2():
                    pkl = cellL["pkl"]
                    rcl = aox.tile([128, 4], F16, tag="rcl")
                    with nc.allow_low_precision(reason="fp16 linv"):
                        nc.vector.reciprocal(rcl[:], pkl[:])
                    linv = aox.tile([1, 512], F16, tag="linv")
                    nc.gpsimd.dma_start(linv[:], rcl[:])
                    lb = scp.tile([128, 512], F32, tag="sc")
                    nc.tensor.matmul(lb[:], onesr[:], linv[:],
                                     start=True, stop=True)
                    lbs = aox.tile([64, 512], F16, tag="lbs")
                    nc.vector.tensor_copy(lbs[:], lb[0:64, :])
                    if hr == 0:
                        dst = ao[hp][ci][0:64, :]
                    else:
                        dst = aox.tile([64, 512], F16, tag="aotmp")
                    nc.vector.tensor_mul(dst, pv[0:64, :], lbs[:])
                    if hr == 1:
                        nc.sync.dma_start(ao[hp][ci][64:128, :], dst)

                cellL = {}
                return groups, lchain1, lchain2

            # ---------- schedule ----------
            # prologue: chunk-0 projections
            proj_k(0)
            proj_v(0)
            trans_v(0)
            proj_q(0, 0)
            proj_q(1, 0)

            projf = deque()   # (due_chunk, closure): before round due_chunk
            pof = deque()     # out-projection fillers, consumed in late rounds

            def pop_filler(allow_po):
                if projf:
                    projf.popleft()[1]()
                    return True
                if pof and allow_po:
                    pof.popleft()()
                    return True
                return False

            for ci in range(NC4):
                while projf and projf[0][0] <= ci:   # safety drain
                    projf.popleft()[1]()
                if ci + 1 < NC4:
                    c = ci + 1
                    projf.append((c, lambda c=c: proj_k(c)))
                    projf.append((c, lambda c=c: proj_v(c)))
                    projf.append((c, lambda c=c: trans_v(c)))
                    projf.append((c, lambda c=c: proj_q(0, c)))
                    projf.append((c, lambda c=c: proj_q(1, c)))
                # build all head group-items for this round
                items = []          # (emit_sc, emit_pv, head)
                lc1s, lc2s = {}, {}
                for h in range(NQH):
                    groups, lc1, lc2 = head_groups(ci, h)
                    lc1s[h], lc2s[h] = lc1, lc2
                    for (es, ep) in groups:
                        items.append((es, ep, h))
                allow_po = ci >= 1
                due = []            # [countdown_items, closure]
                for i, (es, ep, h) in enumerate(items):
                    es()
                    if i > 0:
                        items[i - 1][1]()            # skewed pv of prev item
                        if items[i - 1][2] != h:     # head boundary crossed
                            hprev = items[i - 1][2]
                            due.append([1, lc1s[hprev]])
                            due.append([4, lc2s[hprev]])
                    fired = False
                    for e in due:
                        e[0] -= 1
                    while due and due[0][0] <= 0:
                        due.pop(0)[1]()
                        fired = True
                    pop_filler(allow_po)
                items[-1][1]()
                for e in due:       # flush in order
                    e[1]()
                lc1s[items[-1][2]]()
                pop_filler(allow_po)
                lc2s[items[-1][2]]()
                for tt in range(4):
                    pof.append(lambda t=ci * 4 + tt: po_tile(t))
            while projf or pof:
                pop_filler(True)

    nc.compile()
    return nc


def make_in_maps(x, freqs_cos, freqs_sin, wq, wk, wv, wo):
    """Host-side sharding + layout prep. Returns per-core input dicts."""
    f16 = np.float16
    x = np.asarray(x, np.float32)
    fc = np.asarray(freqs_cos, np.float32)
    fs = np.asarray(freqs_sin, np.float32)
    wq = np.asarray(wq, np.float32)
    wk = np.asarray(wk, np.float32)
    wv = np.asarray(wv, np.float32)
    wo = np.asarray(wo, np.float32)

    perm = np.concatenate([np.arange(0, HD, 2), np.arange(1, HD, 2)])
    cosT = np.ascontiguousarray(fc.T)            # (32, T)
    sinT = np.ascontiguousarray(fs.T)
    cosf = np.concatenate([cosT] * 4, axis=0).astype(f16)    # (128, T)
    sinf = np.concatenate([-sinT, sinT, -sinT, sinT], axis=0).astype(f16)

    jj = np.arange(128)[:, None]
    cc_ = np.arange(128)[None, :]
    tri = (jj <= cc_).astype(f16)                # [key j, query c]
    iden = np.eye(64, dtype=f16)

    def pad128(w):  # (D, 64) -> (D, 128)
        z = np.zeros((D, 128), f16)
        z[:, 0:HD] = w
        return z

    in_maps = []
    for c in range(N_CORES):
        b, g = divmod(c, 4)
        wq_c = wq[:, g * QCOLS:(g + 1) * QCOLS]
        wq_c = np.ascontiguousarray(
            wq_c.reshape(D, NQH, HD)[:, :, perm].reshape(D, QCOLS)).astype(f16)
        wk_c = pad128(wk[:, g * HD:(g + 1) * HD][:, perm].astype(f16))
        wv_c = pad128(wv[:, g * HD:(g + 1) * HD].astype(f16))
        wo_c = np.ascontiguousarray(wo[g * QCOLS:(g + 1) * QCOLS, :]).astype(f16)
        xT_c = np.ascontiguousarray(x[b].T).astype(f16)
        in_maps.append({
            "xT": xT_c, "wq": wq_c, "wk": wk_c, "wv": wv_c, "wo": wo_c,
            "cosf": cosf, "sinf": sinf, "tri": tri, "iden": iden,
        })
    return in_maps


def run_on_cores(in_maps, trace=False, **kwargs):
    if "nc" not in _cache:
        _cache["nc"] = build_nc()
    return run_bass_kernel_spmd(
        _cache["nc"], in_maps, core_ids=list(range(N_CORES)), trace=trace,
        **kwargs)


def kernel(x, freqs_cos, freqs_sin, wq, wk, wv, wo):
    in_maps = make_in_maps(x, freqs_cos, freqs_sin, wq, wk, wv, wo)
    res = run_on_cores(in_maps)
    outs = [np.asarray(res.results[c]["out"], np.float32)
            for c in range(N_CORES)]
    full = np.empty((B, T, D), np.float32)
    for b in range(B):
        full[b] = outs[4 * b] + outs[4 * b + 1] + outs[4 * b + 2] + outs[4 * b + 3]
    return full


# revision 13
# speedup vs baseline: 1.5073x; 1.0711x over previous
"""GQA attention kernel for Trainium2, 8 NeuronCores — fp16, software-pipelined.

Problem: B=2, T=2048, D=1024, 16 Q heads / 4 KV heads, head_dim=64, RoPE,
causal softmax, out-projection.

Sharding: 8 cores = 2 (batch) x 4 (KV group). Core c handles batch c//4 and
KV group g=c%4 (query heads 4g..4g+3). wq/wk/wv column-sharded, wo
row-sharded; the 4 partial outputs per batch are summed on the host.

All matmul operands are fp16 (PSUM accumulates fp32); rel err ~6e-4 vs the
fp64 reference. Every stationary operand is padded to 128 columns so Fast
Weight Load triggers and LDWEIGHTS hides under the previous matmul.

Layout is transposed (head_dim on partitions): xT (D,T), qT (256,T),
kT (64,T dup'd to 128), scoresT[j,i] = k_j.q_i. Softmax computes
exp(s/8 - 4): the bias keeps unnormalized weights in fp16 range and cancels
exactly through 1/L. Causality: the diagonal 512x512 region of each query
chunk is computed TRIMMED — the four 128-key blocks only cover query
columns [128r:512), and all four residual triangles are the SAME [128,128]
0/1 matrix, applied multiplicatively to the fp16 `at` tile after exp
(split over DVE and GpSimd; GpSimd cannot touch PSUM). L rides the PV
matmul as a ones-column of v; 1/L is computed lane-parallel by packing the
L row [1,512] into [128,4] via SBUF-SBUF DMA, then broadcast back over 64
partitions with a ones-stationary matmul.

Scheduling: engines are strict-FIFO, so emission order is the schedule.
The main loop runs query-chunk rounds (ci-outer, heads inner) and weaves
"filler" PE work — next round's K/V/Q projection chunks, previous round's
output-projection tiles — between attention groups, so the PE queue never
drains while ACT grinds exp (ACT is the attention-phase pacer at
(N+352)/1.2 ns). A drained PE triggers the HAM clock gate (PE drops
2.4 -> 1.2 GHz), which is what made previous versions 2x slow. Each head's
L-chain/normalize is deferred into the next head's stretch so its DMA
round-trips never block the ACT/DVE queues.
"""

import numpy as np
import sys
from collections import deque

sys.path.insert(0, "/opt/trn_rl_repo")

from concourse import bass, bacc, mybir, tile  # noqa: E402
from concourse.bass_utils import run_bass_kernel_spmd  # noqa: E402

F32 = mybir.dt.float32
F16 = mybir.dt.float16
EXP = mybir.ActivationFunctionType.Exp

B, T, D = 2, 2048, 1024
HD = 64                      # head dim
NQH = 4                      # query heads per core
QCOLS = NQH * HD             # 256
KC = D // 128                # 8 contraction chunks
NT = T // 128                # 16 row tiles
NC4 = T // 512               # 4 512-wide column chunks
N_CORES = 8

# trimmed diagonal geometry: block r covers query cols [TRIM_OFF[r]:512),
# packed into diag-a (r0,r1,r3) + diag-b (r2) exp groups.
TRIM_OFF = [0, 128, 256, 384]

_cache = {}


def build_nc():
    nc = bacc.Bacc("TRN2", target_bir_lowering=False, debug=False)

    xT_d = nc.declare_dram_parameter("xT", [D, T], F16, isOutput=False)
    wq_d = nc.declare_dram_parameter("wq", [D, QCOLS], F16, isOutput=False)
    wk_d = nc.declare_dram_parameter("wk", [D, 128], F16, isOutput=False)
    wv_d = nc.declare_dram_parameter("wv", [D, 128], F16, isOutput=False)
    wo_d = nc.declare_dram_parameter("wo", [QCOLS, D], F16, isOutput=False)
    cos_d = nc.declare_dram_parameter("cosf", [128, T], F16, isOutput=False)
    sin_d = nc.declare_dram_parameter("sinf", [128, T], F16, isOutput=False)
    tri_d = nc.declare_dram_parameter("tri", [128, 128], F16, isOutput=False)
    idn_d = nc.declare_dram_parameter("iden", [64, 64], F16, isOutput=False)
    out_d = nc.declare_dram_parameter("out", [T, D], F16, isOutput=True)

    with tile.TileContext(nc) as tc:
        with (
            tc.tile_pool(name="sb", bufs=1) as sb,
            tc.tile_pool(name="sbx", bufs=1) as sbx,
            tc.tile_pool(name="rope", bufs=2) as rope_pool,
            tc.tile_pool(name="at", bufs=3) as at_pool,
            tc.tile_pool(name="aox", bufs=2) as aox,
            tc.tile_pool(name="outp", bufs=3) as outp,
            tc.tile_pool(name="wrk", bufs=1, space="PSUM") as wrk,
            tc.tile_pool(name="scp", bufs=2, space="PSUM") as scp,
            tc.tile_pool(name="pvp", bufs=2, space="PSUM") as pvp,
        ):
            wq = sb.tile([128, KC, QCOLS], F16, tag="wq")
            wk = sb.tile([128, KC, 128], F16, tag="wk")
            wv = sb.tile([128, KC, 128], F16, tag="wv")
            wo = sb.tile([128, 2, D], F16, tag="wo")
            cosf = sb.tile([128, T], F16, tag="cosf")
            sinf = sb.tile([128, T], F16, tag="sinf")
            tri = sb.tile([128, 128], F16, tag="tri")
            iden = sb.tile([64, 64], F16, tag="iden")
            onesr = sb.tile([1, 128], F16, tag="onesr")
            bias = sb.tile([128, 1], F32, tag="bias")
            # per-512-chunk tiles (chunk-grain independence for the pipeline)
            qTc = [[sb.tile([128, 512], F16, tag=f"qT{hp}_{ci}",
                            name=f"qT{hp}_{ci}") for ci in range(NC4)]
                   for hp in range(2)]
            kTc = [sb.tile([128, 512], F16, tag=f"kT{ci}", name=f"kT{ci}")
                   for ci in range(NC4)]
            vTc = [sb.tile([64, 512], F16, tag=f"vT{ci}", name=f"vT{ci}")
                   for ci in range(NC4)]
            vc = [sb.tile([128, 4, 128], F16, tag=f"v{ci}", name=f"v{ci}")
                  for ci in range(NC4)]
            ao = [[sb.tile([128, 512], F16, tag=f"ao{hp}_{ci}",
                           name=f"ao{hp}_{ci}") for ci in range(NC4)]
                  for hp in range(2)]
            xT = sbx.tile([128, KC, T], F16, tag="xT")

            nc.gpsimd.memset(onesr[:], 1.0)
            nc.gpsimd.memset(bias[:], -4.0)
            for ci in range(NC4):
                nc.gpsimd.memset(vc[ci][:], 0.0)
                nc.gpsimd.memset(vc[ci][:, :, HD:HD + 1], 1.0)

            for k in range(KC):
                nc.sync.dma_start(xT[:, k, :], xT_d[k * 128:(k + 1) * 128, :])
            for k in range(KC):
                nc.sync.dma_start(wk[:, k, :], wk_d[k * 128:(k + 1) * 128, :])
                nc.sync.dma_start(wv[:, k, :], wv_d[k * 128:(k + 1) * 128, :])
                nc.sync.dma_start(wq[:, k, :], wq_d[k * 128:(k + 1) * 128, :])
            nc.sync.dma_start(cosf[:], cos_d[:])
            nc.sync.dma_start(sinf[:], sin_d[:])
            nc.sync.dma_start(tri[:], tri_d[:])
            nc.sync.dma_start(iden[:], idn_d[:])
            for c in range(2):
                nc.sync.dma_start(wo[:, c, :], wo_d[c * 128:(c + 1) * 128, :])

            # ---------- emission helpers ----------
            def rope_chunk(q_ap, nrows, cs):
                """in-place RoPE on a [*, 512] chunk tile (cs indexes cos/sin)."""
                rot = rope_pool.tile([128, 512], F16, tag="rot", bufs=4)
                for blk in range(nrows // 64):
                    r0 = blk * 64
                    nc.sync.dma_start(rot[r0:r0 + 32, :],
                                      q_ap[r0 + 32:r0 + 64, :])
                    nc.sync.dma_start(rot[r0 + 32:r0 + 64, :],
                                      q_ap[r0:r0 + 32, :])
                nc.vector.tensor_mul(q_ap[0:nrows, :], q_ap[0:nrows, :],
                                     cosf[0:nrows, cs])
                nc.vector.tensor_mul(rot[0:nrows, :], rot[0:nrows, :],
                                     sinf[0:nrows, cs])
                nc.vector.tensor_add(q_ap[0:nrows, :], q_ap[0:nrows, :],
                                     rot[0:nrows, :])

            def proj_k(ci):
                cs = slice(ci * 512, (ci + 1) * 512)
                p = wrk.tile([128, 2, 512], F32, tag="wk_")
                for k in range(KC):
                    nc.tensor.matmul(p[:, 0, :], wk[:, k, :], xT[:, k, cs],
                                     start=(k == 0), stop=(k == KC - 1))
                nc.vector.tensor_copy(kTc[ci][0:64, :], p[0:64, 0, :])
                rope_chunk(kTc[ci], 64, cs)
                nc.sync.dma_start(kTc[ci][64:128, :], kTc[ci][0:64, :])

            def proj_v(ci):
                cs = slice(ci * 512, (ci + 1) * 512)
                p = wrk.tile([128, 2, 512], F32, tag="wk_")
                for k in range(KC):
                    nc.tensor.matmul(p[:, 0, :], wv[:, k, :], xT[:, k, cs],
                                     start=(k == 0), stop=(k == KC - 1))
                nc.vector.tensor_copy(vTc[ci][0:64, :], p[0:64, 0, :])

            def trans_v(ci):
                vtr = wrk.tile([128, 4, 64], F16, tag="wk_")
                for tt in range(4):
                    nc.tensor.transpose(vtr[:, tt, :],
                                        vTc[ci][:, tt * 128:(tt + 1) * 128],
                                        iden[:])
                    nc.vector.tensor_copy(vc[ci][:, tt, 0:HD], vtr[:, tt, :])

            def proj_q(hp, ci):
                cs = slice(ci * 512, (ci + 1) * 512)
                p = wrk.tile([128, 2, 512], F32, tag="wk_")
                for k in range(KC):
                    nc.tensor.matmul(
                        p[:, 0, :], wq[:, k, hp * 128:(hp + 1) * 128],
                        xT[:, k, cs], start=(k == 0), stop=(k == KC - 1))
                nc.vector.tensor_copy(qTc[hp][ci][:], p[:, 0, :])
                rope_chunk(qTc[hp][ci], 128, cs)

            def po_tile(t):
                ci, tt = divmod(t, 4)
                toff = slice(tt * 128, (tt + 1) * 128)
                po = wrk.tile([128, 2, 512], F32, tag="wk_")
                for nh in range(2):
                    ns = slice(nh * 512, (nh + 1) * 512)
                    for cc in range(2):
                        nc.tensor.matmul(
                            po[:, nh, :], ao[cc][ci][:, toff], wo[:, cc, ns],
                            start=(cc == 0), stop=(cc == 1))
                ot = outp.tile([128, D], F16, tag="ot")
                nc.vector.tensor_copy(ot[:, 0:512], po[:, 0, :])
                nc.vector.tensor_copy(ot[:, 512:1024], po[:, 1, :])
                nc.sync.dma_start(out_d[t * 128:(t + 1) * 128, :], ot[:])

            def kchunk(tj):
                """[64 or 128, 128] stationary slice for key tile tj."""
                return kTc[tj // 4], (tj % 4) * 128

            def attention_head(ci, h):
                """Scores+exp+mask+PV for head h, query chunk ci.
                Returns the deferred 1/L + normalize closure."""
                hp, hr = divmod(h, 2)
                qrow = slice(hr * 64, hr * 64 + 64)
                qt = qTc[hp][ci]
                pv = pvp.tile([128, 512], F32, tag="pv")
                n_off = ci * 4
                for tg in range(n_off // 2):
                    scg = scp.tile([128, 2, 512], F32, tag="sc")
                    for j in range(2):
                        tj = tg * 2 + j
                        kt, ko = kchunk(tj)
                        nc.tensor.matmul(
                            scg[:, j, :], kt[qrow, ko:ko + 128], qt[qrow, :],
                            start=True, stop=True)
                    atg = at_pool.tile([128, 2, 512], F16, tag="at")
                    nc.scalar.activation(atg[:], scg[:], EXP,
                                         scale=0.125, bias=bias[:])
                    for j in range(2):
                        tj = tg * 2 + j
                        nc.tensor.matmul(pv[:], vc[tj // 4][:, tj % 4, :],
                                         atg[:, j, :],
                                         start=(tj == 0), stop=False)
                # diagonal, trimmed: r0@[0:512], r1@[512:896], r3@[896:1024]
                kt, _ = kchunk(n_off)
                sca = scp.tile([128, 2, 512], F32, tag="sc")
                nc.tensor.matmul(sca[:, 0, :], kt[qrow, 0:128],
                                 qt[qrow, :], start=True, stop=True)
                nc.tensor.matmul(sca[:, 1, 0:384], kt[qrow, 128:256],
                                 qt[qrow, 128:512], start=True, stop=True)
                nc.tensor.matmul(sca[:, 1, 384:512], kt[qrow, 384:512],
                                 qt[qrow, 384:512], start=True, stop=True)
                ata = at_pool.tile([128, 2, 512], F16, tag="at")
                nc.scalar.activation(ata[:], sca[:], EXP,
                                     scale=0.125, bias=bias[:])
                scb = scp.tile([128, 2, 512], F32, tag="sc")
                nc.tensor.matmul(scb[:, 0, 0:256], kt[qrow, 256:384],
                                 qt[qrow, 256:512], start=True, stop=True)
                atb = at_pool.tile([128, 256], F16, tag="at")
                nc.scalar.activation(atb[:], scb[:, 0, 0:256], EXP,
                                     scale=0.125, bias=bias[:])
                nc.vector.tensor_mul(ata[:, 0, 0:128], ata[:, 0, 0:128],
                                     tri[:])
                nc.gpsimd.tensor_mul(ata[:, 1, 0:128], ata[:, 1, 0:128],
                                     tri[:])
                nc.gpsimd.tensor_mul(ata[:, 1, 384:512], ata[:, 1, 384:512],
                                     tri[:])
                nc.vector.tensor_mul(atb[:, 0:128], atb[:, 0:128], tri[:])
                vd = vc[ci]
                nc.tensor.matmul(pv[:, 0:512], vd[:, 0, :], ata[:, 0, :],
                                 start=(n_off == 0), stop=False)
                nc.tensor.matmul(pv[:, 128:512], vd[:, 1, :], ata[:, 1, 0:384],
                                 start=False, stop=False)
                nc.tensor.matmul(pv[:, 384:512], vd[:, 3, :],
                                 ata[:, 1, 384:512], start=False, stop=False)
                nc.tensor.matmul(pv[:, 256:512], vd[:, 2, :], atb[:],
                                 start=False, stop=True)

                def lchain():
                    lrow = aox.tile([1, 512], F32, tag="lrow")
                    nc.scalar.copy(lrow[:], pv[64:65, :])
                    pkl = aox.tile([128, 4], F32, tag="pkl")
                    nc.gpsimd.dma_start(pkl[:], lrow[:])
                    rcl = aox.tile([128, 4], F16, tag="rcl")
                    with nc.allow_low_precision(reason="fp16 linv"):
                        nc.vector.reciprocal(rcl[:], pkl[:])
                    linv = aox.tile([1, 512], F16, tag="linv")
                    nc.gpsimd.dma_start(linv[:], rcl[:])
                    lb = scp.tile([128, 512], F32, tag="sc")
                    nc.tensor.matmul(lb[:], onesr[:], linv[:],
                                     start=True, stop=True)
                    lbs = aox.tile([64, 512], F16, tag="lbs")
                    nc.vector.tensor_copy(lbs[:], lb[0:64, :])
                    if hr == 0:
                        dst = ao[hp][ci][0:64, :]
                    else:
                        dst = aox.tile([64, 512], F16, tag="aotmp")
                    nc.vector.tensor_mul(dst, pv[0:64, :], lbs[:])
                    if hr == 1:
                        nc.sync.dma_start(ao[hp][ci][64:128, :], dst)
                return lchain

            # ---------- schedule ----------
            proj_k(0)
            proj_v(0)
            trans_v(0)
            proj_q(0, 0)
            proj_q(1, 0)

            projf = deque()   # (due_chunk, closure)
            pof = deque()     # out-projection fillers: drained in round 3

            def pop_filler(allow_po):
                if projf:
                    projf.popleft()[1]()
                    return True
                if pof and allow_po:
                    pof.popleft()()
                    return True
                return False

            pending = None
            for ci in range(NC4):
                while projf and projf[0][0] <= ci:   # safety drain
                    projf.popleft()[1]()
                if ci + 1 < NC4:
                    c = ci + 1
                    projf.append((c, lambda c=c: proj_k(c)))
                    projf.append((c, lambda c=c: proj_v(c)))
                    projf.append((c, lambda c=c: trans_v(c)))
                    projf.append((c, lambda c=c: proj_q(0, c)))
                    projf.append((c, lambda c=c: proj_q(1, c)))
                allow_po = ci == NC4 - 1
                for h in range(NQH):
                    nxt = attention_head(ci, h)
                    if pending is not None:
                        pending()
                    pending = nxt
                    for _ in range(3 if allow_po else 2):
                        pop_filler(allow_po)
                pending()
                pending = None
                for tt in range(4):
                    pof.append(lambda t=ci * 4 + tt: po_tile(t))
            while projf or pof:
                pop_filler(True)

    nc.compile()
    return nc


def make_in_maps(x, freqs_cos, freqs_sin, wq, wk, wv, wo):
    """Host-side sharding + layout prep. Returns per-core input dicts."""
    f16 = np.float16
    x = np.asarray(x, np.float32)
    fc = np.asarray(freqs_cos, np.float32)
    fs = np.asarray(freqs_sin, np.float32)
    wq = np.asarray(wq, np.float32)
    wk = np.asarray(wk, np.float32)
    wv = np.asarray(wv, np.float32)
    wo = np.asarray(wo, np.float32)

    perm = np.concatenate([np.arange(0, HD, 2), np.arange(1, HD, 2)])
    cosT = np.ascontiguousarray(fc.T)            # (32, T)
    sinT = np.ascontiguousarray(fs.T)
    cosf = np.concatenate([cosT] * 4, axis=0).astype(f16)    # (128, T)
    sinf = np.concatenate([-sinT, sinT, -sinT, sinT], axis=0).astype(f16)

    jj = np.arange(128)[:, None]
    cc_ = np.arange(128)[None, :]
    tri = (jj <= cc_).astype(f16)                # [key j, query c]
    iden = np.eye(64, dtype=f16)

    def pad128(w):  # (D, 64) -> (D, 128)
        z = np.zeros((D, 128), f16)
        z[:, 0:HD] = w
        return z

    in_maps = []
    for c in range(N_CORES):
        b, g = divmod(c, 4)
        wq_c = wq[:, g * QCOLS:(g + 1) * QCOLS]
        wq_c = np.ascontiguousarray(
            wq_c.reshape(D, NQH, HD)[:, :, perm].reshape(D, QCOLS)).astype(f16)
        wk_c = pad128(wk[:, g * HD:(g + 1) * HD][:, perm].astype(f16))
        wv_c = pad128(wv[:, g * HD:(g + 1) * HD].astype(f16))
        wo_c = np.ascontiguousarray(wo[g * QCOLS:(g + 1) * QCOLS, :]).astype(f16)
        xT_c = np.ascontiguousarray(x[b].T).astype(f16)
        in_maps.append({
            "xT": xT_c, "wq": wq_c, "wk": wk_c, "wv": wv_c, "wo": wo_c,
            "cosf": cosf, "sinf": sinf, "tri": tri, "iden": iden,
        })
    return in_maps


def run_on_cores(in_maps, trace=False, **kwargs):
    if "nc" not in _cache:
        _cache["nc"] = build_nc()
    return run_bass_kernel_spmd(
        _cache["nc"], in_maps, core_ids=list(range(N_CORES)), trace=trace,
        **kwargs)


def kernel(x, freqs_cos, freqs_sin, wq, wk, wv, wo):
    in_maps = make_in_maps(x, freqs_cos, freqs_sin, wq, wk, wv, wo)
    res = run_on_cores(in_maps)
    outs = [np.asarray(res.results[c]["out"], np.float32)
            for c in range(N_CORES)]
    full = np.empty((B, T, D), np.float32)
    for b in range(B):
        full[b] = outs[4 * b] + outs[4 * b + 1] + outs[4 * b + 2] + outs[4 * b + 3]
    return full


# revision 14
# speedup vs baseline: 1.5515x; 1.0293x over previous
"""GQA attention kernel for Trainium2, 8 NeuronCores — fp16, software-pipelined.

Problem: B=2, T=2048, D=1024, 16 Q heads / 4 KV heads, head_dim=64, RoPE,
causal softmax, out-projection.

Sharding: 8 cores = 2 (batch) x 4 (KV group). Core c handles batch c//4 and
KV group g=c%4 (query heads 4g..4g+3). wq/wk/wv column-sharded, wo
row-sharded; the 4 partial outputs per batch are summed on the host.

All matmul operands are fp16 (PSUM accumulates fp32); rel err ~6e-4 vs the
fp64 reference. Every stationary operand is padded to 128 columns so Fast
Weight Load triggers and LDWEIGHTS hides under the previous matmul.

Layout is transposed (head_dim on partitions): xT (D,T), qT (256,T),
kT (64,T dup'd to 128), scoresT[j,i] = k_j.q_i. Softmax computes
exp(s/8 - 4): the bias keeps unnormalized weights in fp16 range and cancels
exactly through 1/L. Causality: the diagonal 512x512 region of each query
chunk is computed TRIMMED — the four 128-key blocks only cover query
columns [128r:512), and all four residual triangles are the SAME [128,128]
0/1 matrix, applied multiplicatively to the fp16 `at` tile after exp
(split over DVE and GpSimd; GpSimd cannot touch PSUM). L rides the PV
matmul as a ones-column of v; 1/L is computed lane-parallel by packing the
L row [1,512] into [128,4] via SBUF-SBUF DMA, then broadcast back over 64
partitions with a ones-stationary matmul.

Scheduling: engines are strict-FIFO, so emission order is the schedule.
The main loop runs query-chunk rounds (ci-outer, heads inner) and weaves
"filler" PE work — next round's K/V/Q projection chunks, previous round's
output-projection tiles — between attention groups, so the PE queue never
drains while ACT grinds exp (ACT is the attention-phase pacer at
(N+352)/1.2 ns). A drained PE triggers the HAM clock gate (PE drops
2.4 -> 1.2 GHz), which is what made previous versions 2x slow. Each head's
L-chain/normalize is deferred into the next head's stretch so its DMA
round-trips never block the ACT/DVE queues.
"""

import numpy as np
import sys
from collections import deque

sys.path.insert(0, "/opt/trn_rl_repo")

from concourse import bass, bacc, mybir, tile  # noqa: E402
from concourse.bass_utils import run_bass_kernel_spmd  # noqa: E402

F32 = mybir.dt.float32
F16 = mybir.dt.float16
EXP = mybir.ActivationFunctionType.Exp

B, T, D = 2, 2048, 1024
HD = 64                      # head dim
NQH = 4                      # query heads per core
QCOLS = NQH * HD             # 256
KC = D // 128                # 8 contraction chunks
NT = T // 128                # 16 row tiles
NC4 = T // 512               # 4 512-wide column chunks
N_CORES = 8

# trimmed diagonal geometry: block r covers query cols [TRIM_OFF[r]:512),
# packed into diag-a (r0,r1,r3) + diag-b (r2) exp groups.
TRIM_OFF = [0, 128, 256, 384]

_cache = {}


def build_nc():
    nc = bacc.Bacc("TRN2", target_bir_lowering=False, debug=False)

    xT_d = nc.declare_dram_parameter("xT", [D, T], F16, isOutput=False)
    wq_d = nc.declare_dram_parameter("wq", [D, QCOLS], F16, isOutput=False)
    wk_d = nc.declare_dram_parameter("wk", [D, 128], F16, isOutput=False)
    wv_d = nc.declare_dram_parameter("wv", [D, 128], F16, isOutput=False)
    wo_d = nc.declare_dram_parameter("wo", [QCOLS, D], F16, isOutput=False)
    cos_d = nc.declare_dram_parameter("cosf", [128, T], F16, isOutput=False)
    sin_d = nc.declare_dram_parameter("sinf", [128, T], F16, isOutput=False)
    tri_d = nc.declare_dram_parameter("tri", [128, 128], F16, isOutput=False)
    idn_d = nc.declare_dram_parameter("iden", [64, 64], F16, isOutput=False)
    out_d = nc.declare_dram_parameter("out", [T, D], F16, isOutput=True)

    with tile.TileContext(nc) as tc:
        with (
            tc.tile_pool(name="sb", bufs=1) as sb,
            tc.tile_pool(name="sbx", bufs=1) as sbx,
            tc.tile_pool(name="rope", bufs=2) as rope_pool,
            tc.tile_pool(name="at", bufs=4) as at_pool,
            tc.tile_pool(name="aox", bufs=3) as aox,
            tc.tile_pool(name="outp", bufs=3) as outp,
            tc.tile_pool(name="wrk", bufs=1, space="PSUM") as wrk,
            tc.tile_pool(name="scp", bufs=2, space="PSUM") as scp,
            tc.tile_pool(name="pvp", bufs=2, space="PSUM") as pvp,
        ):
            wq = sb.tile([128, KC, QCOLS], F16, tag="wq")
            wk = sb.tile([128, KC, 128], F16, tag="wk")
            wv = sb.tile([128, KC, 128], F16, tag="wv")
            wo = sb.tile([128, 2, D], F16, tag="wo")
            cosf = sb.tile([128, T], F16, tag="cosf")
            sinf = sb.tile([128, T], F16, tag="sinf")
            tri = sb.tile([128, 128], F16, tag="tri")
            iden = sb.tile([64, 64], F16, tag="iden")
            onesr = sb.tile([1, 128], F16, tag="onesr")
            bias = sb.tile([128, 1], F32, tag="bias")
            # per-512-chunk tiles (chunk-grain independence for the pipeline)
            qTc = [[sb.tile([128, 512], F16, tag=f"qT{hp}_{ci}",
                            name=f"qT{hp}_{ci}") for ci in range(NC4)]
                   for hp in range(2)]
            kTc = [sb.tile([128, 512], F16, tag=f"kT{ci}", name=f"kT{ci}")
                   for ci in range(NC4)]
            vTc = [sb.tile([64, 512], F16, tag=f"vT{ci}", name=f"vT{ci}")
                   for ci in range(NC4)]
            vc = [sb.tile([128, 4, 128], F16, tag=f"v{ci}", name=f"v{ci}")
                  for ci in range(NC4)]
            ao = [[sb.tile([128, 512], F16, tag=f"ao{hp}_{ci}",
                           name=f"ao{hp}_{ci}") for ci in range(NC4)]
                  for hp in range(2)]
            xT = sbx.tile([128, KC, T], F16, tag="xT")

            nc.gpsimd.memset(onesr[:], 1.0)
            nc.gpsimd.memset(bias[:], -4.0)
            for ci in range(NC4):
                nc.gpsimd.memset(vc[ci][:], 0.0)
                nc.gpsimd.memset(vc[ci][:, :, HD:HD + 1], 1.0)

            # input DMAs: round-0-critical data first, column-sliced so the
            # first projections don't wait for the whole 4MB of xT.
            for k in range(KC):
                nc.sync.dma_start(wk[:, k, :], wk_d[k * 128:(k + 1) * 128, :])
                nc.sync.dma_start(wv[:, k, :], wv_d[k * 128:(k + 1) * 128, :])
            for k in range(KC):
                nc.sync.dma_start(xT[:, k, 0:512],
                                  xT_d[k * 128:(k + 1) * 128, 0:512])
            nc.sync.dma_start(cosf[:, 0:512], cos_d[:, 0:512])
            nc.sync.dma_start(sinf[:, 0:512], sin_d[:, 0:512])
            nc.sync.dma_start(tri[:], tri_d[:])
            nc.sync.dma_start(iden[:], idn_d[:])
            for k in range(KC):
                nc.sync.dma_start(wq[:, k, :], wq_d[k * 128:(k + 1) * 128, :])
            for ci in range(1, NC4):
                cs = slice(ci * 512, (ci + 1) * 512)
                for k in range(KC):
                    nc.gpsimd.dma_start(xT[:, k, cs],
                                        xT_d[k * 128:(k + 1) * 128, cs])
                nc.gpsimd.dma_start(cosf[:, cs], cos_d[:, cs])
                nc.gpsimd.dma_start(sinf[:, cs], sin_d[:, cs])
            for c in range(2):
                nc.gpsimd.dma_start(wo[:, c, :], wo_d[c * 128:(c + 1) * 128, :])

            # ---------- emission helpers ----------
            def rope_chunk(q_ap, nrows, cs):
                """in-place RoPE on a [*, 512] chunk tile (cs indexes cos/sin)."""
                rot = rope_pool.tile([128, 512], F16, tag="rot", bufs=4)
                for blk in range(nrows // 64):
                    r0 = blk * 64
                    nc.sync.dma_start(rot[r0:r0 + 32, :],
                                      q_ap[r0 + 32:r0 + 64, :])
                    nc.sync.dma_start(rot[r0 + 32:r0 + 64, :],
                                      q_ap[r0:r0 + 32, :])
                nc.vector.tensor_mul(q_ap[0:nrows, :], q_ap[0:nrows, :],
                                     cosf[0:nrows, cs])
                nc.vector.tensor_mul(rot[0:nrows, :], rot[0:nrows, :],
                                     sinf[0:nrows, cs])
                nc.vector.tensor_add(q_ap[0:nrows, :], q_ap[0:nrows, :],
                                     rot[0:nrows, :])

            def proj_k(ci):
                cs = slice(ci * 512, (ci + 1) * 512)
                p = wrk.tile([128, 2, 512], F32, tag="wk_")
                for k in range(KC):
                    nc.tensor.matmul(p[:, 0, :], wk[:, k, :], xT[:, k, cs],
                                     start=(k == 0), stop=(k == KC - 1))
                nc.vector.tensor_copy(kTc[ci][0:64, :], p[0:64, 0, :])
                rope_chunk(kTc[ci], 64, cs)
                nc.sync.dma_start(kTc[ci][64:128, :], kTc[ci][0:64, :])

            def proj_v(ci):
                cs = slice(ci * 512, (ci + 1) * 512)
                p = wrk.tile([128, 2, 512], F32, tag="wk_")
                for k in range(KC):
                    nc.tensor.matmul(p[:, 0, :], wv[:, k, :], xT[:, k, cs],
                                     start=(k == 0), stop=(k == KC - 1))
                nc.vector.tensor_copy(vTc[ci][0:64, :], p[0:64, 0, :])

            def trans_v(ci):
                vtr = wrk.tile([128, 4, 64], F16, tag="wk_")
                for tt in range(4):
                    nc.tensor.transpose(vtr[:, tt, :],
                                        vTc[ci][:, tt * 128:(tt + 1) * 128],
                                        iden[:])
                    nc.vector.tensor_copy(vc[ci][:, tt, 0:HD], vtr[:, tt, :])

            def proj_q(hp, ci):
                cs = slice(ci * 512, (ci + 1) * 512)
                p = wrk.tile([128, 2, 512], F32, tag="wk_")
                for k in range(KC):
                    nc.tensor.matmul(
                        p[:, 0, :], wq[:, k, hp * 128:(hp + 1) * 128],
                        xT[:, k, cs], start=(k == 0), stop=(k == KC - 1))
                nc.vector.tensor_copy(qTc[hp][ci][:], p[:, 0, :])
                rope_chunk(qTc[hp][ci], 128, cs)

            def po_tile(t):
                ci, tt = divmod(t, 4)
                toff = slice(tt * 128, (tt + 1) * 128)
                po = wrk.tile([128, 2, 512], F32, tag="wk_")
                for nh in range(2):
                    ns = slice(nh * 512, (nh + 1) * 512)
                    for cc in range(2):
                        nc.tensor.matmul(
                            po[:, nh, :], ao[cc][ci][:, toff], wo[:, cc, ns],
                            start=(cc == 0), stop=(cc == 1))
                ot = outp.tile([128, D], F16, tag="ot")
                nc.vector.tensor_copy(ot[:, 0:512], po[:, 0, :])
                nc.sync.dma_start(out_d[t * 128:(t + 1) * 128, 0:512],
                                  ot[:, 0:512])
                nc.vector.tensor_copy(ot[:, 512:1024], po[:, 1, :])
                nc.sync.dma_start(out_d[t * 128:(t + 1) * 128, 512:1024],
                                  ot[:, 512:1024])

            def kchunk(tj):
                """[64 or 128, 128] stationary slice for key tile tj."""
                return kTc[tj // 4], (tj % 4) * 128

            def attention_head(ci, h):
                """Scores+exp+mask+PV for head h, query chunk ci.
                Returns the deferred 1/L + normalize closure."""
                hp, hr = divmod(h, 2)
                qrow = slice(hr * 64, hr * 64 + 64)
                qt = qTc[hp][ci]
                pv = pvp.tile([128, 512], F32, tag="pv")
                n_off = ci * 4
                for tg in range(n_off // 2):
                    scg = scp.tile([128, 2, 512], F32, tag="sc")
                    for j in range(2):
                        tj = tg * 2 + j
                        kt, ko = kchunk(tj)
                        nc.tensor.matmul(
                            scg[:, j, :], kt[qrow, ko:ko + 128], qt[qrow, :],
                            start=True, stop=True)
                    atg = at_pool.tile([128, 2, 512], F16, tag="at")
                    nc.scalar.activation(atg[:], scg[:], EXP,
                                         scale=0.125, bias=bias[:])
                    for j in range(2):
                        tj = tg * 2 + j
                        nc.tensor.matmul(pv[:], vc[tj // 4][:, tj % 4, :],
                                         atg[:, j, :],
                                         start=(tj == 0), stop=False)
                # diagonal, trimmed: r0@[0:512], r1@[512:896], r3@[896:1024]
                kt, _ = kchunk(n_off)
                sca = scp.tile([128, 2, 512], F32, tag="sc")
                nc.tensor.matmul(sca[:, 0, :], kt[qrow, 0:128],
                                 qt[qrow, :], start=True, stop=True)
                nc.tensor.matmul(sca[:, 1, 0:384], kt[qrow, 128:256],
                                 qt[qrow, 128:512], start=True, stop=True)
                nc.tensor.matmul(sca[:, 1, 384:512], kt[qrow, 384:512],
                                 qt[qrow, 384:512], start=True, stop=True)
                ata = at_pool.tile([128, 2, 512], F16, tag="at")
                nc.scalar.activation(ata[:], sca[:], EXP,
                                     scale=0.125, bias=bias[:])
                scb = scp.tile([128, 2, 512], F32, tag="sc")
                nc.tensor.matmul(scb[:, 0, 0:256], kt[qrow, 256:384],
                                 qt[qrow, 256:512], start=True, stop=True)
                atb = at_pool.tile([128, 256], F16, tag="at")
                nc.scalar.activation(atb[:], scb[:, 0, 0:256], EXP,
                                     scale=0.125, bias=bias[:])
                nc.vector.tensor_mul(ata[:, 0, 0:128], ata[:, 0, 0:128],
                                     tri[:])
                nc.gpsimd.tensor_mul(ata[:, 1, 0:128], ata[:, 1, 0:128],
                                     tri[:])
                nc.gpsimd.tensor_mul(ata[:, 1, 384:512], ata[:, 1, 384:512],
                                     tri[:])
                nc.vector.tensor_mul(atb[:, 0:128], atb[:, 0:128], tri[:])
                vd = vc[ci]
                nc.tensor.matmul(pv[:, 0:512], vd[:, 0, :], ata[:, 0, :],
                                 start=(n_off == 0), stop=False)
                nc.tensor.matmul(pv[:, 128:512], vd[:, 1, :], ata[:, 1, 0:384],
                                 start=False, stop=False)
                nc.tensor.matmul(pv[:, 384:512], vd[:, 3, :],
                                 ata[:, 1, 384:512], start=False, stop=False)
                nc.tensor.matmul(pv[:, 256:512], vd[:, 2, :], atb[:],
                                 start=False, stop=True)

                def lchain():
                    lrow = aox.tile([1, 512], F32, tag="lrow")
                    nc.scalar.copy(lrow[:], pv[64:65, :])
                    pkl = aox.tile([128, 4], F32, tag="pkl")
                    nc.gpsimd.dma_start(pkl[:], lrow[:])
                    rcl = aox.tile([128, 4], F16, tag="rcl")
                    with nc.allow_low_precision(reason="fp16 linv"):
                        nc.vector.reciprocal(rcl[:], pkl[:])
                    linv = aox.tile([1, 512], F16, tag="linv")
                    nc.gpsimd.dma_start(linv[:], rcl[:])
                    lb = scp.tile([128, 512], F32, tag="sc")
                    nc.tensor.matmul(lb[:], onesr[:], linv[:],
                                     start=True, stop=True)
                    lbs = aox.tile([64, 512], F16, tag="lbs")
                    nc.vector.tensor_copy(lbs[:], lb[0:64, :])
                    if hr == 0:
                        dst = ao[hp][ci][0:64, :]
                    else:
                        dst = aox.tile([64, 512], F16, tag="aotmp")
                    nc.vector.tensor_mul(dst, pv[0:64, :], lbs[:])
                    if hr == 1:
                        nc.sync.dma_start(ao[hp][ci][64:128, :], dst)
                return lchain

            # ---------- schedule ----------
            proj_k(0)
            proj_v(0)
            trans_v(0)
            proj_q(0, 0)
            proj_q(1, 0)

            projf = deque()   # (due_chunk, closure)
            pof = deque()     # out-projection fillers: drained in round 3

            def pop_filler(allow_po):
                if projf:
                    projf.popleft()[1]()
                    return True
                if pof and allow_po:
                    pof.popleft()()
                    return True
                return False

            pending = None
            for ci in range(NC4):
                while projf and projf[0][0] <= ci:   # safety drain
                    projf.popleft()[1]()
                if ci + 1 < NC4:
                    c = ci + 1
                    projf.append((c, lambda c=c: proj_k(c)))
                    projf.append((c, lambda c=c: proj_v(c)))
                    projf.append((c, lambda c=c: trans_v(c)))
                    projf.append((c, lambda c=c: proj_q(0, c)))
                    projf.append((c, lambda c=c: proj_q(1, c)))
                allow_po = ci == NC4 - 1
                for h in range(NQH):
                    nxt = attention_head(ci, h)
                    if pending is not None:
                        pending()
                    pending = nxt
                    for _ in range(3 if allow_po else 2):
                        pop_filler(allow_po)
                pending()
                pending = None
                for tt in range(4):
                    pof.append(lambda t=ci * 4 + tt: po_tile(t))
            while projf or pof:
                pop_filler(True)

    nc.compile()
    return nc


def make_in_maps(x, freqs_cos, freqs_sin, wq, wk, wv, wo):
    """Host-side sharding + layout prep. Returns per-core input dicts."""
    f16 = np.float16
    x = np.asarray(x, np.float32)
    fc = np.asarray(freqs_cos, np.float32)
    fs = np.asarray(freqs_sin, np.float32)
    wq = np.asarray(wq, np.float32)
    wk = np.asarray(wk, np.float32)
    wv = np.asarray(wv, np.float32)
    wo = np.asarray(wo, np.float32)

    perm = np.concatenate([np.arange(0, HD, 2), np.arange(1, HD, 2)])
    cosT = np.ascontiguousarray(fc.T)            # (32, T)
    sinT = np.ascontiguousarray(fs.T)
    cosf = np.concatenate([cosT] * 4, axis=0).astype(f16)    # (128, T)
    sinf = np.concatenate([-sinT, sinT, -sinT, sinT], axis=0).astype(f16)

    jj = np.arange(128)[:, None]
    cc_ = np.arange(128)[None, :]
    tri = (jj <= cc_).astype(f16)                # [key j, query c]
    iden = np.eye(64, dtype=f16)

    def pad128(w):  # (D, 64) -> (D, 128)
        z = np.zeros((D, 128), f16)
        z[:, 0:HD] = w
        return z

    in_maps = []
    for c in range(N_CORES):
        b, g = divmod(c, 4)
        wq_c = wq[:, g * QCOLS:(g + 1) * QCOLS]
        wq_c = np.ascontiguousarray(
            wq_c.reshape(D, NQH, HD)[:, :, perm].reshape(D, QCOLS)).astype(f16)
        wk_c = pad128(wk[:, g * HD:(g + 1) * HD][:, perm].astype(f16))
        wv_c = pad128(wv[:, g * HD:(g + 1) * HD].astype(f16))
        wo_c = np.ascontiguousarray(wo[g * QCOLS:(g + 1) * QCOLS, :]).astype(f16)
        xT_c = np.ascontiguousarray(x[b].T).astype(f16)
        in_maps.append({
            "xT": xT_c, "wq": wq_c, "wk": wk_c, "wv": wv_c, "wo": wo_c,
            "cosf": cosf, "sinf": sinf, "tri": tri, "iden": iden,
        })
    return in_maps


def run_on_cores(in_maps, trace=False, **kwargs):
    if "nc" not in _cache:
        _cache["nc"] = build_nc()
    return run_bass_kernel_spmd(
        _cache["nc"], in_maps, core_ids=list(range(N_CORES)), trace=trace,
        **kwargs)


def kernel(x, freqs_cos, freqs_sin, wq, wk, wv, wo):
    in_maps = make_in_maps(x, freqs_cos, freqs_sin, wq, wk, wv, wo)
    res = run_on_cores(in_maps)
    outs = [np.asarray(res.results[c]["out"], np.float32)
            for c in range(N_CORES)]
    full = np.empty((B, T, D), np.float32)
    for b in range(B):
        full[b] = outs[4 * b] + outs[4 * b + 1] + outs[4 * b + 2] + outs[4 * b + 3]
    return full


# revision 16
# speedup vs baseline: 1.5955x; 1.0284x over previous
"""GQA attention kernel for Trainium2, 8 NeuronCores — fp16, software-pipelined.

Problem: B=2, T=2048, D=1024, 16 Q heads / 4 KV heads, head_dim=64, RoPE,
causal softmax, out-projection.

Sharding: 8 cores = 2 (batch) x 4 (KV group). Core c handles batch c//4 and
KV group g=c%4 (query heads 4g..4g+3). wq/wk/wv column-sharded, wo
row-sharded; the 4 partial outputs per batch are summed on the host.

All matmul operands are fp16 (PSUM accumulates fp32); rel err ~6e-4 vs the
fp64 reference. Every stationary operand is padded to 128 columns so Fast
Weight Load triggers and LDWEIGHTS hides under the previous matmul.

Layout is transposed (head_dim on partitions): xT (D,T), qT (256,T),
kT (64,T dup'd to 128), scoresT[j,i] = k_j.q_i. Softmax computes
exp(s/8 - 4): the bias keeps unnormalized weights in fp16 range and cancels
exactly through 1/L. Causality: the diagonal 512x512 region of each query
chunk is computed TRIMMED — the four 128-key blocks only cover query
columns [128r:512), and all four residual triangles are the SAME [128,128]
0/1 matrix, applied multiplicatively to the fp16 `at` tile after exp
(split over DVE and GpSimd; GpSimd cannot touch PSUM). L rides the PV
matmul as a ones-column of v; 1/L is computed lane-parallel by packing the
L row [1,512] into [128,4] via SBUF-SBUF DMA, then broadcast back over 64
partitions with a ones-stationary matmul.

Scheduling: engines are strict-FIFO, so emission order is the schedule.
The main loop runs query-chunk rounds (ci-outer, heads inner) and weaves
"filler" PE work — next round's K/V/Q projection chunks, previous round's
output-projection tiles — between attention groups, so the PE queue never
drains while ACT grinds exp (ACT is the attention-phase pacer at
(N+352)/1.2 ns). A drained PE triggers the HAM clock gate (PE drops
2.4 -> 1.2 GHz), which is what made previous versions 2x slow. Each head's
L-chain/normalize is deferred into the next head's stretch so its DMA
round-trips never block the ACT/DVE queues.
"""

import numpy as np
import sys
from collections import deque

sys.path.insert(0, "/opt/trn_rl_repo")

from concourse import bass, bacc, mybir, tile  # noqa: E402
from concourse.bass_utils import run_bass_kernel_spmd  # noqa: E402

F32 = mybir.dt.float32
F16 = mybir.dt.float16
EXP = mybir.ActivationFunctionType.Exp

B, T, D = 2, 2048, 1024
HD = 64                      # head dim
NQH = 4                      # query heads per core
QCOLS = NQH * HD             # 256
KC = D // 128                # 8 contraction chunks
NT = T // 128                # 16 row tiles
NC4 = T // 512               # 4 512-wide column chunks
N_CORES = 8

# trimmed diagonal geometry: block r covers query cols [TRIM_OFF[r]:512),
# packed into diag-a (r0,r1,r3) + diag-b (r2) exp groups.
TRIM_OFF = [0, 128, 256, 384]

_cache = {}


def build_nc():
    nc = bacc.Bacc("TRN2", target_bir_lowering=False, debug=False)

    xT_d = nc.declare_dram_parameter("xT", [D, T], F16, isOutput=False)
    wq_d = nc.declare_dram_parameter("wq", [D, QCOLS], F16, isOutput=False)
    wk_d = nc.declare_dram_parameter("wk", [D, 128], F16, isOutput=False)
    wv_d = nc.declare_dram_parameter("wv", [D, 128], F16, isOutput=False)
    wo_d = nc.declare_dram_parameter("wo", [QCOLS, D], F16, isOutput=False)
    cos_d = nc.declare_dram_parameter("cosf", [128, T], F16, isOutput=False)
    sin_d = nc.declare_dram_parameter("sinf", [128, T], F16, isOutput=False)
    tri_d = nc.declare_dram_parameter("tri", [128, 128], F16, isOutput=False)
    idn_d = nc.declare_dram_parameter("iden", [64, 64], F16, isOutput=False)
    out_d = nc.declare_dram_parameter("out", [T, D], F16, isOutput=True)

    with tile.TileContext(nc) as tc:
        with (
            tc.tile_pool(name="sb", bufs=1) as sb,
            tc.tile_pool(name="sbx", bufs=1) as sbx,
            tc.tile_pool(name="rope", bufs=2) as rope_pool,
            tc.tile_pool(name="at", bufs=4) as at_pool,
            tc.tile_pool(name="aox", bufs=3) as aox,
            tc.tile_pool(name="outp", bufs=3) as outp,
            tc.tile_pool(name="wrk", bufs=1, space="PSUM") as wrk,
            tc.tile_pool(name="scp", bufs=2, space="PSUM") as scp,
            tc.tile_pool(name="pvp", bufs=2, space="PSUM") as pvp,
        ):
            wq = sb.tile([128, KC, QCOLS], F16, tag="wq")
            wk = sb.tile([128, KC, 128], F16, tag="wk")
            wv = sb.tile([128, KC, 128], F16, tag="wv")
            wo = sb.tile([128, 2, D], F16, tag="wo")
            cosf = sb.tile([128, T], F16, tag="cosf")
            sinf = sb.tile([128, T], F16, tag="sinf")
            tri = sb.tile([128, 128], F16, tag="tri")
            iden = sb.tile([64, 64], F16, tag="iden")
            onesr = sb.tile([1, 128], F16, tag="onesr")
            bias = sb.tile([128, 1], F32, tag="bias")
            # per-512-chunk tiles (chunk-grain independence for the pipeline)
            qTc = [[sb.tile([128, 512], F16, tag=f"qT{hp}_{ci}",
                            name=f"qT{hp}_{ci}") for ci in range(NC4)]
                   for hp in range(2)]
            kTc = [sb.tile([128, 512], F16, tag=f"kT{ci}", name=f"kT{ci}")
                   for ci in range(NC4)]
            vTc = [sb.tile([64, 512], F16, tag=f"vT{ci}", name=f"vT{ci}")
                   for ci in range(NC4)]
            vc = [sb.tile([128, 4, 128], F16, tag=f"v{ci}", name=f"v{ci}")
                  for ci in range(NC4)]
            ao = [[sb.tile([128, 512], F16, tag=f"ao{hp}_{ci}",
                           name=f"ao{hp}_{ci}") for ci in range(NC4)]
                  for hp in range(2)]
            xT = sbx.tile([128, KC, T], F16, tag="xT")

            nc.gpsimd.memset(onesr[:], 1.0)
            nc.gpsimd.memset(bias[:], -4.0)
            for ci in range(NC4):
                nc.gpsimd.memset(vc[ci][:], 0.0)
                nc.gpsimd.memset(vc[ci][:, :, HD:HD + 1], 1.0)

            # input DMAs: round-0-critical data first, column-sliced, and
            # issued from four different engine queues in parallel (a single
            # queue pays ~565ns of issue time per dma_start).
            for k in range(KC):
                nc.sync.dma_start(wk[:, k, :], wk_d[k * 128:(k + 1) * 128, :])
                nc.gpsimd.dma_start(xT[:, k, 0:512],
                                    xT_d[k * 128:(k + 1) * 128, 0:512])
                nc.scalar.dma_start(wv[:, k, :], wv_d[k * 128:(k + 1) * 128, :])
            nc.sync.dma_start(cosf[:, 0:512], cos_d[:, 0:512])
            nc.sync.dma_start(sinf[:, 0:512], sin_d[:, 0:512])
            nc.sync.dma_start(tri[:], tri_d[:])
            nc.sync.dma_start(iden[:], idn_d[:])
            for k in range(KC):
                nc.sync.dma_start(wq[:, k, :], wq_d[k * 128:(k + 1) * 128, :])
            for ci in range(1, NC4):
                cs = slice(ci * 512, (ci + 1) * 512)
                for k in range(KC):
                    nc.gpsimd.dma_start(xT[:, k, cs],
                                        xT_d[k * 128:(k + 1) * 128, cs])
                nc.gpsimd.dma_start(cosf[:, cs], cos_d[:, cs])
                nc.gpsimd.dma_start(sinf[:, cs], sin_d[:, cs])
            for c in range(2):
                nc.gpsimd.dma_start(wo[:, c, :], wo_d[c * 128:(c + 1) * 128, :])

            # ---------- emission helpers ----------
            def rope_chunk(q_ap, nrows, cs):
                """in-place RoPE on a [*, 512] chunk tile (cs indexes cos/sin)."""
                rot = rope_pool.tile([128, 512], F16, tag="rot", bufs=4)
                for blk in range(nrows // 64):
                    r0 = blk * 64
                    nc.sync.dma_start(rot[r0:r0 + 32, :],
                                      q_ap[r0 + 32:r0 + 64, :])
                    nc.sync.dma_start(rot[r0 + 32:r0 + 64, :],
                                      q_ap[r0:r0 + 32, :])
                nc.vector.tensor_mul(q_ap[0:nrows, :], q_ap[0:nrows, :],
                                     cosf[0:nrows, cs])
                nc.vector.tensor_mul(rot[0:nrows, :], rot[0:nrows, :],
                                     sinf[0:nrows, cs])
                nc.vector.tensor_add(q_ap[0:nrows, :], q_ap[0:nrows, :],
                                     rot[0:nrows, :])

            def proj_k(ci):
                cs = slice(ci * 512, (ci + 1) * 512)
                p = wrk.tile([128, 2, 512], F32, tag="wk_")
                for k in range(KC):
                    nc.tensor.matmul(p[:, 0, :], wk[:, k, :], xT[:, k, cs],
                                     start=(k == 0), stop=(k == KC - 1))
                nc.vector.tensor_copy(kTc[ci][0:64, :], p[0:64, 0, :])
                rope_chunk(kTc[ci], 64, cs)
                nc.sync.dma_start(kTc[ci][64:128, :], kTc[ci][0:64, :])

            def proj_v(ci):
                cs = slice(ci * 512, (ci + 1) * 512)
                p = wrk.tile([128, 2, 512], F32, tag="wk_")
                for k in range(KC):
                    nc.tensor.matmul(p[:, 0, :], wv[:, k, :], xT[:, k, cs],
                                     start=(k == 0), stop=(k == KC - 1))
                nc.vector.tensor_copy(vTc[ci][0:64, :], p[0:64, 0, :])

            def trans_v(ci):
                vtr = wrk.tile([128, 4, 64], F16, tag="wk_")
                for tt in range(4):
                    nc.tensor.transpose(vtr[:, tt, :],
                                        vTc[ci][:, tt * 128:(tt + 1) * 128],
                                        iden[:])
                    nc.vector.tensor_copy(vc[ci][:, tt, 0:HD], vtr[:, tt, :])

            def proj_q(hp, ci):
                cs = slice(ci * 512, (ci + 1) * 512)
                p = wrk.tile([128, 2, 512], F32, tag="wk_")
                for k in range(KC):
                    nc.tensor.matmul(
                        p[:, 0, :], wq[:, k, hp * 128:(hp + 1) * 128],
                        xT[:, k, cs], start=(k == 0), stop=(k == KC - 1))
                nc.vector.tensor_copy(qTc[hp][ci][:], p[:, 0, :])
                rope_chunk(qTc[hp][ci], 128, cs)

            def po_tile(t, tail=False):
                ci, tt = divmod(t, 4)
                toff = slice(tt * 128, (tt + 1) * 128)
                if tail:
                    po = scp.tile([128, 2, 512], F32, tag="sc")
                else:
                    po = wrk.tile([128, 2, 512], F32, tag="wk_")
                for nh in range(2):
                    ns = slice(nh * 512, (nh + 1) * 512)
                    for cc in range(2):
                        nc.tensor.matmul(
                            po[:, nh, :], ao[cc][ci][:, toff], wo[:, cc, ns],
                            start=(cc == 0), stop=(cc == 1))
                ot = outp.tile([128, D], F16, tag="ot")
                nc.vector.tensor_copy(ot[:, 0:512], po[:, 0, :])
                nc.sync.dma_start(out_d[t * 128:(t + 1) * 128, 0:512],
                                  ot[:, 0:512])
                nc.vector.tensor_copy(ot[:, 512:1024], po[:, 1, :])
                nc.sync.dma_start(out_d[t * 128:(t + 1) * 128, 512:1024],
                                  ot[:, 512:1024])

            def kchunk(tj):
                """[64 or 128, 128] stationary slice for key tile tj."""
                return kTc[tj // 4], (tj % 4) * 128

            def attention_head(ci, h):
                """Scores+exp+mask+PV for head h, query chunk ci.
                Returns the deferred 1/L + normalize closure."""
                hp, hr = divmod(h, 2)
                qrow = slice(hr * 64, hr * 64 + 64)
                qt = qTc[hp][ci]
                pv = pvp.tile([128, 512], F32, tag="pv")
                n_off = ci * 4
                for tg in range(n_off // 2):
                    scg = scp.tile([128, 2, 512], F32, tag="sc")
                    for j in range(2):
                        tj = tg * 2 + j
                        kt, ko = kchunk(tj)
                        nc.tensor.matmul(
                            scg[:, j, :], kt[qrow, ko:ko + 128], qt[qrow, :],
                            start=True, stop=True)
                    atg = at_pool.tile([128, 2, 512], F16, tag="at")
                    nc.scalar.activation(atg[:], scg[:], EXP,
                                         scale=0.125, bias=bias[:])
                    for j in range(2):
                        tj = tg * 2 + j
                        nc.tensor.matmul(pv[:], vc[tj // 4][:, tj % 4, :],
                                         atg[:, j, :],
                                         start=(tj == 0), stop=False)
                # diagonal, trimmed: r0@[0:512], r1@[512:896], r3@[896:1024]
                kt, _ = kchunk(n_off)
                sca = scp.tile([128, 2, 512], F32, tag="sc")
                nc.tensor.matmul(sca[:, 0, :], kt[qrow, 0:128],
                                 qt[qrow, :], start=True, stop=True)
                nc.tensor.matmul(sca[:, 1, 0:384], kt[qrow, 128:256],
                                 qt[qrow, 128:512], start=True, stop=True)
                nc.tensor.matmul(sca[:, 1, 384:512], kt[qrow, 384:512],
                                 qt[qrow, 384:512], start=True, stop=True)
                ata = at_pool.tile([128, 2, 512], F16, tag="at")
                nc.scalar.activation(ata[:], sca[:], EXP,
                                     scale=0.125, bias=bias[:])
                scb = scp.tile([128, 2, 512], F32, tag="sc")
                nc.tensor.matmul(scb[:, 0, 0:256], kt[qrow, 256:384],
                                 qt[qrow, 256:512], start=True, stop=True)
                atb = at_pool.tile([128, 256], F16, tag="at")
                nc.scalar.activation(atb[:], scb[:, 0, 0:256], EXP,
                                     scale=0.125, bias=bias[:])
                nc.vector.tensor_mul(ata[:, 0, 0:128], ata[:, 0, 0:128],
                                     tri[:])
                nc.gpsimd.tensor_mul(ata[:, 1, 0:128], ata[:, 1, 0:128],
                                     tri[:])
                nc.gpsimd.tensor_mul(ata[:, 1, 384:512], ata[:, 1, 384:512],
                                     tri[:])
                nc.vector.tensor_mul(atb[:, 0:128], atb[:, 0:128], tri[:])
                vd = vc[ci]
                nc.tensor.matmul(pv[:, 0:512], vd[:, 0, :], ata[:, 0, :],
                                 start=(n_off == 0), stop=False)
                nc.tensor.matmul(pv[:, 128:512], vd[:, 1, :], ata[:, 1, 0:384],
                                 start=False, stop=False)
                nc.tensor.matmul(pv[:, 384:512], vd[:, 3, :],
                                 ata[:, 1, 384:512], start=False, stop=False)
                nc.tensor.matmul(pv[:, 256:512], vd[:, 2, :], atb[:],
                                 start=False, stop=True)

                def lchain():
                    lrow = aox.tile([1, 512], F32, tag="lrow")
                    nc.scalar.copy(lrow[:], pv[64:65, :])
                    pkl = aox.tile([128, 4], F32, tag="pkl")
                    nc.gpsimd.dma_start(pkl[:], lrow[:])
                    rcl = aox.tile([128, 4], F16, tag="rcl")
                    with nc.allow_low_precision(reason="fp16 linv"):
                        nc.vector.reciprocal(rcl[:], pkl[:])
                    linv = aox.tile([1, 512], F16, tag="linv")
                    nc.gpsimd.dma_start(linv[:], rcl[:])
                    lb = scp.tile([128, 512], F32, tag="sc")
                    nc.tensor.matmul(lb[:], onesr[:], linv[:],
                                     start=True, stop=True)
                    lbs = aox.tile([64, 512], F16, tag="lbs")
                    nc.vector.tensor_copy(lbs[:], lb[0:64, :])
                    if hr == 0:
                        dst = ao[hp][ci][0:64, :]
                    else:
                        dst = aox.tile([64, 512], F16, tag="aotmp")
                    nc.vector.tensor_mul(dst, pv[0:64, :], lbs[:])
                    if hr == 1:
                        nc.sync.dma_start(ao[hp][ci][64:128, :], dst)
                return lchain

            # ---------- schedule ----------
            proj_k(0)
            proj_v(0)
            trans_v(0)
            proj_q(0, 0)
            proj_q(1, 0)

            projf = deque()   # (due_chunk, closure)
            pof = deque()     # out-projection fillers: drained in round 3

            def pop_filler(allow_po):
                if projf:
                    projf.popleft()[1]()
                    return True
                if pof and allow_po:
                    pof.popleft()()
                    return True
                return False

            pending = None
            for ci in range(NC4):
                while projf and projf[0][0] <= ci:   # safety drain
                    projf.popleft()[1]()
                for c in ([1, 2] if ci == 0 else [3] if ci == 1 else []):
                    projf.append((c, lambda c=c: proj_k(c)))
                    projf.append((c, lambda c=c: proj_v(c)))
                    projf.append((c, lambda c=c: trans_v(c)))
                    projf.append((c, lambda c=c: proj_q(0, c)))
                    projf.append((c, lambda c=c: proj_q(1, c)))
                allow_po = ci >= 2
                n_pop = 3 if ci == 0 else 2
                for h in range(NQH):
                    nxt = attention_head(ci, h)
                    if pending is not None:
                        pending()
                    pending = nxt
                    for _ in range(n_pop):
                        pop_filler(allow_po)
                pending()
                pending = None
                if ci < NC4 - 1:
                    for tt in range(4):
                        pof.append(lambda t=ci * 4 + tt: po_tile(t))
            while projf:
                projf.popleft()[1]()
            while pof:
                pof.popleft()()
            # tail: the sc/pv PSUM banks are free now — ping-pong the last
            # out-projection tiles through the sc slots so MMs overlap evacs.
            for tt in range(4):
                po_tile((NC4 - 1) * 4 + tt, tail=True)

    nc.compile()
    return nc


def make_in_maps(x, freqs_cos, freqs_sin, wq, wk, wv, wo):
    """Host-side sharding + layout prep. Returns per-core input dicts."""
    f16 = np.float16
    x = np.asarray(x, np.float32)
    fc = np.asarray(freqs_cos, np.float32)
    fs = np.asarray(freqs_sin, np.float32)
    wq = np.asarray(wq, np.float32)
    wk = np.asarray(wk, np.float32)
    wv = np.asarray(wv, np.float32)
    wo = np.asarray(wo, np.float32)

    perm = np.concatenate([np.arange(0, HD, 2), np.arange(1, HD, 2)])
    cosT = np.ascontiguousarray(fc.T)            # (32, T)
    sinT = np.ascontiguousarray(fs.T)
    cosf = np.concatenate([cosT] * 4, axis=0).astype(f16)    # (128, T)
    sinf = np.concatenate([-sinT, sinT, -sinT, sinT], axis=0).astype(f16)

    jj = np.arange(128)[:, None]
    cc_ = np.arange(128)[None, :]
    tri = (jj <= cc_).astype(f16)                # [key j, query c]
    iden = np.eye(64, dtype=f16)

    def pad128(w):  # (D, 64) -> (D, 128)
        z = np.zeros((D, 128), f16)
        z[:, 0:HD] = w
        return z

    in_maps = []
    for c in range(N_CORES):
        b, g = divmod(c, 4)
        wq_c = wq[:, g * QCOLS:(g + 1) * QCOLS]
        wq_c = np.ascontiguousarray(
            wq_c.reshape(D, NQH, HD)[:, :, perm].reshape(D, QCOLS)).astype(f16)
        wk_c = pad128(wk[:, g * HD:(g + 1) * HD][:, perm].astype(f16))
        wv_c = pad128(wv[:, g * HD:(g + 1) * HD].astype(f16))
        wo_c = np.ascontiguousarray(wo[g * QCOLS:(g + 1) * QCOLS, :]).astype(f16)
        xT_c = np.ascontiguousarray(x[b].T).astype(f16)
        in_maps.append({
            "xT": xT_c, "wq": wq_c, "wk": wk_c, "wv": wv_c, "wo": wo_c,
            "cosf": cosf, "sinf": sinf, "tri": tri, "iden": iden,
        })
    return in_maps


def run_on_cores(in_maps, trace=False, **kwargs):
    if "nc" not in _cache:
        _cache["nc"] = build_nc()
    return run_bass_kernel_spmd(
        _cache["nc"], in_maps, core_ids=list(range(N_CORES)), trace=trace,
        **kwargs)


def kernel(x, freqs_cos, freqs_sin, wq, wk, wv, wo):
    in_maps = make_in_maps(x, freqs_cos, freqs_sin, wq, wk, wv, wo)
    res = run_on_cores(in_maps)
    outs = [np.asarray(res.results[c]["out"], np.float32)
            for c in range(N_CORES)]
    full = np.empty((B, T, D), np.float32)
    for b in range(B):
        full[b] = outs[4 * b] + outs[4 * b + 1] + outs[4 * b + 2] + outs[4 * b + 3]
    return full


# revision 17
# speedup vs baseline: 1.7604x; 1.1033x over previous
"""GQA attention kernel for Trainium2, 8 NeuronCores — fp16, software-pipelined.

Problem: B=2, T=2048, D=1024, 16 Q heads / 4 KV heads, head_dim=64, RoPE,
causal softmax, out-projection.

Sharding: 8 cores = 2 (batch) x 4 (KV group). Core c handles batch c//4 and
KV group g=c%4 (query heads 4g..4g+3). wq/wk/wv column-sharded, wo
row-sharded; the 4 partial outputs per batch are summed on the host.

All matmul operands are fp16 (PSUM accumulates fp32); rel err ~6e-4 vs the
fp64 reference. Every stationary operand is padded to 128 columns so Fast
Weight Load triggers and LDWEIGHTS hides under the previous matmul.

Layout is transposed (head_dim on partitions): xT (D,T), qT (256,T),
kT (64,T dup'd to 128), scoresT[j,i] = k_j.q_i. Softmax computes
exp(s/8 - 4): the bias keeps unnormalized weights in fp16 range and cancels
exactly through 1/L. Causality: the diagonal 512x512 region of each query
chunk is computed TRIMMED — the four 128-key blocks only cover query
columns [128r:512), and all four residual triangles are the SAME [128,128]
0/1 matrix, applied multiplicatively to the fp16 `at` tile after exp
(split over DVE and GpSimd; GpSimd cannot touch PSUM). L rides the PV
matmul as a ones-column of v; 1/L is computed lane-parallel by packing the
L row [1,512] into [128,4] via SBUF-SBUF DMA, then broadcast back over 64
partitions with a ones-stationary matmul.

Scheduling: engines are strict-FIFO, so emission order is the schedule.
The main loop runs query-chunk rounds (ci-outer, heads inner) and weaves
"filler" PE work — next round's K/V/Q projection chunks, previous round's
output-projection tiles — between attention groups, so the PE queue never
drains while ACT grinds exp (ACT is the attention-phase pacer at
(N+352)/1.2 ns). A drained PE triggers the HAM clock gate (PE drops
2.4 -> 1.2 GHz), which is what made previous versions 2x slow. Each head's
L-chain/normalize is deferred into the next head's stretch so its DMA
round-trips never block the ACT/DVE queues.
"""

import numpy as np
import sys
from collections import deque

sys.path.insert(0, "/opt/trn_rl_repo")

from concourse import bass, bacc, mybir, tile  # noqa: E402
from concourse.bass_utils import run_bass_kernel_spmd  # noqa: E402

F32 = mybir.dt.float32
F16 = mybir.dt.float16
EXP = mybir.ActivationFunctionType.Exp

B, T, D = 2, 2048, 1024
HD = 64                      # head dim
NQH = 4                      # query heads per core
QCOLS = NQH * HD             # 256
KC = D // 128                # 8 contraction chunks
NT = T // 128                # 16 row tiles
NC4 = T // 512               # 4 512-wide column chunks
N_CORES = 8

# trimmed diagonal geometry: block r covers query cols [TRIM_OFF[r]:512),
# packed into diag-a (r0,r1,r3) + diag-b (r2) exp groups.
TRIM_OFF = [0, 128, 256, 384]

_cache = {}


def build_nc():
    nc = bacc.Bacc("TRN2", target_bir_lowering=False, debug=False)

    xT_d = nc.declare_dram_parameter("xT", [D, T], F16, isOutput=False)
    wq_d = nc.declare_dram_parameter("wq", [D, QCOLS], F16, isOutput=False)
    wk_d = nc.declare_dram_parameter("wk", [D, 128], F16, isOutput=False)
    wv_d = nc.declare_dram_parameter("wv", [D, 128], F16, isOutput=False)
    wo_d = nc.declare_dram_parameter("wo", [QCOLS, D], F16, isOutput=False)
    cos_d = nc.declare_dram_parameter("cosf", [128, T], F16, isOutput=False)
    sin_d = nc.declare_dram_parameter("sinf", [128, T], F16, isOutput=False)
    tri_d = nc.declare_dram_parameter("tri", [128, 128], F16, isOutput=False)
    idn_d = nc.declare_dram_parameter("iden", [64, 64], F16, isOutput=False)
    out_d = nc.declare_dram_parameter("out", [T, D], F16, isOutput=True)

    with tile.TileContext(nc) as tc:
        with (
            tc.tile_pool(name="sb", bufs=1) as sb,
            tc.tile_pool(name="sbx", bufs=1) as sbx,
            tc.tile_pool(name="rope", bufs=2) as rope_pool,
            tc.tile_pool(name="at", bufs=4) as at_pool,
            tc.tile_pool(name="aox", bufs=3) as aox,
            tc.tile_pool(name="outp", bufs=3) as outp,
            tc.tile_pool(name="wrk", bufs=1, space="PSUM") as wrk,
            tc.tile_pool(name="scp", bufs=2, space="PSUM") as scp,
            tc.tile_pool(name="pvp", bufs=2, space="PSUM") as pvp,
        ):
            wq = sb.tile([128, KC, QCOLS], F16, tag="wq")
            wk = sb.tile([128, KC, 128], F16, tag="wk")
            wv = sb.tile([128, KC, 128], F16, tag="wv")
            wo = sb.tile([128, 2, D], F16, tag="wo")
            cosf = sb.tile([128, T], F16, tag="cosf")
            sinf = sb.tile([128, T], F16, tag="sinf")
            tri = sb.tile([128, 128], F16, tag="tri")
            iden = sb.tile([64, 64], F16, tag="iden")
            onesr = sb.tile([1, 128], F16, tag="onesr")
            bias = sb.tile([128, 1], F32, tag="bias")
            # per-512-chunk tiles (chunk-grain independence for the pipeline)
            qTc = [[sb.tile([128, 512], F16, tag=f"qT{hp}_{ci}",
                            name=f"qT{hp}_{ci}") for ci in range(NC4)]
                   for hp in range(2)]
            kTc = [sb.tile([128, 512], F16, tag=f"kT{ci}", name=f"kT{ci}")
                   for ci in range(NC4)]
            vTc = [sb.tile([64, 512], F16, tag=f"vT{ci}", name=f"vT{ci}")
                   for ci in range(NC4)]
            vc = [sb.tile([128, 4, 128], F16, tag=f"v{ci}", name=f"v{ci}")
                  for ci in range(NC4)]
            ao = [[sb.tile([128, 512], F16, tag=f"ao{hp}_{ci}",
                           name=f"ao{hp}_{ci}") for ci in range(NC4)]
                  for hp in range(2)]
            xT = sbx.tile([128, KC, T], F16, tag="xT")


            # input DMAs: round-0-critical data first, column-sliced, and
            # issued from four different engine queues in parallel (a single
            # queue pays ~565ns of issue time per dma_start).
            for k in range(KC):
                nc.sync.dma_start(wk[:, k, :], wk_d[k * 128:(k + 1) * 128, :])
                nc.gpsimd.dma_start(xT[:, k, 0:512],
                                    xT_d[k * 128:(k + 1) * 128, 0:512])
                nc.scalar.dma_start(wv[:, k, :], wv_d[k * 128:(k + 1) * 128, :])
            nc.gpsimd.memset(onesr[:], 1.0)
            nc.gpsimd.memset(bias[:], -4.0)
            for ci in range(NC4):
                nc.gpsimd.memset(vc[ci][:], 0.0)
                nc.gpsimd.memset(vc[ci][:, :, HD:HD + 1], 1.0)
            nc.sync.dma_start(cosf[:, 0:512], cos_d[:, 0:512])
            nc.sync.dma_start(sinf[:, 0:512], sin_d[:, 0:512])
            nc.sync.dma_start(tri[:], tri_d[:])
            nc.sync.dma_start(iden[:], idn_d[:])
            for k in range(KC):
                nc.sync.dma_start(wq[:, k, :], wq_d[k * 128:(k + 1) * 128, :])
            for ci in range(1, NC4):
                cs = slice(ci * 512, (ci + 1) * 512)
                for k in range(KC):
                    nc.gpsimd.dma_start(xT[:, k, cs],
                                        xT_d[k * 128:(k + 1) * 128, cs])
                nc.gpsimd.dma_start(cosf[:, cs], cos_d[:, cs])
                nc.gpsimd.dma_start(sinf[:, cs], sin_d[:, cs])
            for c in range(2):
                nc.gpsimd.dma_start(wo[:, c, :], wo_d[c * 128:(c + 1) * 128, :])

            # ---------- emission helpers ----------
            def rope_chunk(q_ap, nrows, cs):
                """in-place RoPE on a [*, 512] chunk tile (cs indexes cos/sin)."""
                rot = rope_pool.tile([128, 512], F16, tag="rot", bufs=4)
                for blk in range(nrows // 64):
                    r0 = blk * 64
                    nc.sync.dma_start(rot[r0:r0 + 32, :],
                                      q_ap[r0 + 32:r0 + 64, :])
                    nc.sync.dma_start(rot[r0 + 32:r0 + 64, :],
                                      q_ap[r0:r0 + 32, :])
                nc.vector.tensor_mul(q_ap[0:nrows, :], q_ap[0:nrows, :],
                                     cosf[0:nrows, cs])
                nc.vector.tensor_mul(rot[0:nrows, :], rot[0:nrows, :],
                                     sinf[0:nrows, cs])
                nc.vector.tensor_add(q_ap[0:nrows, :], q_ap[0:nrows, :],
                                     rot[0:nrows, :])

            def proj_k(ci):
                cs = slice(ci * 512, (ci + 1) * 512)
                p = wrk.tile([128, 2, 512], F32, tag="wk_")
                for k in range(KC):
                    nc.tensor.matmul(p[:, 0, :], wk[:, k, :], xT[:, k, cs],
                                     start=(k == 0), stop=(k == KC - 1))
                nc.vector.tensor_copy(kTc[ci][0:64, :], p[0:64, 0, :])
                rope_chunk(kTc[ci], 64, cs)
                nc.sync.dma_start(kTc[ci][64:128, :], kTc[ci][0:64, :])

            def proj_v(ci):
                cs = slice(ci * 512, (ci + 1) * 512)
                p = wrk.tile([128, 2, 512], F32, tag="wk_")
                for k in range(KC):
                    nc.tensor.matmul(p[:, 0, :], wv[:, k, :], xT[:, k, cs],
                                     start=(k == 0), stop=(k == KC - 1))
                nc.vector.tensor_copy(vTc[ci][0:64, :], p[0:64, 0, :])

            def trans_v(ci):
                vtr = wrk.tile([128, 4, 64], F16, tag="wk_")
                for tt in range(4):
                    nc.tensor.transpose(vtr[:, tt, :],
                                        vTc[ci][:, tt * 128:(tt + 1) * 128],
                                        iden[:])
                    nc.vector.tensor_copy(vc[ci][:, tt, 0:HD], vtr[:, tt, :])

            def proj_q(hp, ci):
                cs = slice(ci * 512, (ci + 1) * 512)
                p = wrk.tile([128, 2, 512], F32, tag="wk_")
                for k in range(KC):
                    nc.tensor.matmul(
                        p[:, 0, :], wq[:, k, hp * 128:(hp + 1) * 128],
                        xT[:, k, cs], start=(k == 0), stop=(k == KC - 1))
                nc.vector.tensor_copy(qTc[hp][ci][:], p[:, 0, :])
                rope_chunk(qTc[hp][ci], 128, cs)

            def po_tile(t, tail=False):
                ci, tt = divmod(t, 4)
                toff = slice(tt * 128, (tt + 1) * 128)
                if tail:
                    po = scp.tile([128, 2, 512], F32, tag="sc")
                else:
                    po = wrk.tile([128, 2, 512], F32, tag="wk_")
                for nh in range(2):
                    ns = slice(nh * 512, (nh + 1) * 512)
                    for cc in range(2):
                        nc.tensor.matmul(
                            po[:, nh, :], ao[cc][ci][:, toff], wo[:, cc, ns],
                            start=(cc == 0), stop=(cc == 1))
                ot = outp.tile([128, D], F16, tag="ot")
                nc.vector.tensor_copy(ot[:, 0:512], po[:, 0, :])
                nc.sync.dma_start(out_d[t * 128:(t + 1) * 128, 0:512],
                                  ot[:, 0:512])
                nc.vector.tensor_copy(ot[:, 512:1024], po[:, 1, :])
                nc.sync.dma_start(out_d[t * 128:(t + 1) * 128, 512:1024],
                                  ot[:, 512:1024])

            def kchunk(tj):
                """[64 or 128, 128] stationary slice for key tile tj."""
                return kTc[tj // 4], (tj % 4) * 128

            def attention_head(ci, h):
                """Scores+exp+mask+PV for head h, query chunk ci.
                Returns the deferred 1/L + normalize closure."""
                hp, hr = divmod(h, 2)
                qrow = slice(hr * 64, hr * 64 + 64)
                qt = qTc[hp][ci]
                pv = pvp.tile([128, 512], F32, tag="pv")
                n_off = ci * 4
                for tg in range(n_off // 2):
                    scg = scp.tile([128, 2, 512], F32, tag="sc")
                    for j in range(2):
                        tj = tg * 2 + j
                        kt, ko = kchunk(tj)
                        nc.tensor.matmul(
                            scg[:, j, :], kt[qrow, ko:ko + 128], qt[qrow, :],
                            start=True, stop=True)
                    atg = at_pool.tile([128, 2, 512], F16, tag="at")
                    nc.scalar.activation(atg[:], scg[:], EXP,
                                         scale=0.125, bias=bias[:])
                    for j in range(2):
                        tj = tg * 2 + j
                        nc.tensor.matmul(pv[:], vc[tj // 4][:, tj % 4, :],
                                         atg[:, j, :],
                                         start=(tj == 0), stop=False)
                # diagonal, trimmed: r0@[0:512], r1@[512:896], r3@[896:1024]
                kt, _ = kchunk(n_off)
                sca = scp.tile([128, 2, 512], F32, tag="sc")
                nc.tensor.matmul(sca[:, 0, :], kt[qrow, 0:128],
                                 qt[qrow, :], start=True, stop=True)
                nc.tensor.matmul(sca[:, 1, 0:384], kt[qrow, 128:256],
                                 qt[qrow, 128:512], start=True, stop=True)
                nc.tensor.matmul(sca[:, 1, 384:512], kt[qrow, 384:512],
                                 qt[qrow, 384:512], start=True, stop=True)
                ata = at_pool.tile([128, 2, 512], F16, tag="at")
                nc.scalar.activation(ata[:], sca[:], EXP,
                                     scale=0.125, bias=bias[:])
                scb = scp.tile([128, 2, 512], F32, tag="sc")
                nc.tensor.matmul(scb[:, 0, 0:256], kt[qrow, 256:384],
                                 qt[qrow, 256:512], start=True, stop=True)
                atb = at_pool.tile([128, 256], F16, tag="at")
                nc.scalar.activation(atb[:], scb[:, 0, 0:256], EXP,
                                     scale=0.125, bias=bias[:])
                nc.vector.tensor_mul(ata[:, 0, 0:128], ata[:, 0, 0:128],
                                     tri[:])
                nc.gpsimd.tensor_mul(ata[:, 1, 0:128], ata[:, 1, 0:128],
                                     tri[:])
                nc.gpsimd.tensor_mul(ata[:, 1, 384:512], ata[:, 1, 384:512],
                                     tri[:])
                nc.vector.tensor_mul(atb[:, 0:128], atb[:, 0:128], tri[:])
                vd = vc[ci]
                nc.tensor.matmul(pv[:, 0:512], vd[:, 0, :], ata[:, 0, :],
                                 start=(n_off == 0), stop=False)
                nc.tensor.matmul(pv[:, 128:512], vd[:, 1, :], ata[:, 1, 0:384],
                                 start=False, stop=False)
                nc.tensor.matmul(pv[:, 384:512], vd[:, 3, :],
                                 ata[:, 1, 384:512], start=False, stop=False)
                nc.tensor.matmul(pv[:, 256:512], vd[:, 2, :], atb[:],
                                 start=False, stop=True)

                def lchain():
                    lrow = aox.tile([1, 512], F32, tag="lrow")
                    nc.scalar.copy(lrow[:], pv[64:65, :])
                    pkl = aox.tile([128, 4], F32, tag="pkl")
                    nc.gpsimd.dma_start(pkl[:], lrow[:])
                    rcl = aox.tile([128, 4], F16, tag="rcl")
                    with nc.allow_low_precision(reason="fp16 linv"):
                        nc.vector.reciprocal(rcl[:], pkl[:])
                    linv = aox.tile([1, 512], F16, tag="linv")
                    nc.gpsimd.dma_start(linv[:], rcl[:])
                    lb = scp.tile([128, 512], F32, tag="sc")
                    nc.tensor.matmul(lb[:], onesr[:], linv[:],
                                     start=True, stop=True)
                    lbs = aox.tile([64, 512], F16, tag="lbs")
                    nc.vector.tensor_copy(lbs[:], lb[0:64, :])
                    if hr == 0:
                        dst = ao[hp][ci][0:64, :]
                    else:
                        dst = aox.tile([64, 512], F16, tag="aotmp")
                    nc.vector.tensor_mul(dst, pv[0:64, :], lbs[:])
                    if hr == 1:
                        nc.sync.dma_start(ao[hp][ci][64:128, :], dst)
                return lchain

            # ---------- schedule ----------
            proj_k(0)
            proj_v(0)
            trans_v(0)
            proj_q(0, 0)
            proj_q(1, 0)

            projf = deque()   # (due_chunk, closure)
            pof = deque()     # out-projection fillers: drained in round 3

            def pop_filler(allow_po):
                if projf:
                    projf.popleft()[1]()
                    return True
                if pof and allow_po:
                    pof.popleft()()
                    return True
                return False

            pending = None
            for ci in range(NC4):
                while projf and projf[0][0] <= ci:   # safety drain
                    projf.popleft()[1]()
                horder = [1, 3, 0, 2] if ci == NC4 - 1 else range(NQH)
                for c in ([1, 2] if ci == 0 else [3] if ci == 1 else []):
                    projf.append((c, lambda c=c: proj_k(c)))
                    projf.append((c, lambda c=c: proj_v(c)))
                    projf.append((c, lambda c=c: trans_v(c)))
                    projf.append((c, lambda c=c: proj_q(0, c)))
                    projf.append((c, lambda c=c: proj_q(1, c)))
                allow_po = ci >= 2
                n_pop = 3 if ci == 0 else 2
                for h in horder:
                    nxt = attention_head(ci, h)
                    if pending is not None:
                        pending()
                    pending = nxt
                    for _ in range(n_pop):
                        pop_filler(allow_po)
                # the last head's L-chain is NOT flushed here: its lb matmul
                # would head-of-line block the next round's scores in the PE
                # FIFO while the L pack/unpack DMAs fly. It fires inside the
                # next round instead (po tiles are consumed >= 2 rounds later).
                if ci < NC4 - 1:
                    for tt in range(4):
                        pof.append(lambda t=ci * 4 + tt: po_tile(t))
            pending()
            pending = None
            while projf:
                projf.popleft()[1]()
            while pof:
                pof.popleft()()
            # tail: the sc/pv PSUM banks are free now — ping-pong the last
            # out-projection tiles through the sc slots so MMs overlap evacs.
            for tt in range(4):
                po_tile((NC4 - 1) * 4 + tt, tail=True)

    nc.compile()
    return nc


def make_in_maps(x, freqs_cos, freqs_sin, wq, wk, wv, wo):
    """Host-side sharding + layout prep. Returns per-core input dicts."""
    f16 = np.float16
    x = np.asarray(x, np.float32)
    fc = np.asarray(freqs_cos, np.float32)
    fs = np.asarray(freqs_sin, np.float32)
    wq = np.asarray(wq, np.float32)
    wk = np.asarray(wk, np.float32)
    wv = np.asarray(wv, np.float32)
    wo = np.asarray(wo, np.float32)

    perm = np.concatenate([np.arange(0, HD, 2), np.arange(1, HD, 2)])
    cosT = np.ascontiguousarray(fc.T)            # (32, T)
    sinT = np.ascontiguousarray(fs.T)
    cosf = np.concatenate([cosT] * 4, axis=0).astype(f16)    # (128, T)
    sinf = np.concatenate([-sinT, sinT, -sinT, sinT], axis=0).astype(f16)

    jj = np.arange(128)[:, None]
    cc_ = np.arange(128)[None, :]
    tri = (jj <= cc_).astype(f16)                # [key j, query c]
    iden = np.eye(64, dtype=f16)

    def pad128(w):  # (D, 64) -> (D, 128)
        z = np.zeros((D, 128), f16)
        z[:, 0:HD] = w
        return z

    in_maps = []
    for c in range(N_CORES):
        b, g = divmod(c, 4)
        wq_c = wq[:, g * QCOLS:(g + 1) * QCOLS]
        wq_c = np.ascontiguousarray(
            wq_c.reshape(D, NQH, HD)[:, :, perm].reshape(D, QCOLS)).astype(f16)
        wk_c = pad128(wk[:, g * HD:(g + 1) * HD][:, perm].astype(f16))
        wv_c = pad128(wv[:, g * HD:(g + 1) * HD].astype(f16))
        wo_c = np.ascontiguousarray(wo[g * QCOLS:(g + 1) * QCOLS, :]).astype(f16)
        xT_c = np.ascontiguousarray(x[b].T).astype(f16)
        in_maps.append({
            "xT": xT_c, "wq": wq_c, "wk": wk_c, "wv": wv_c, "wo": wo_c,
            "cosf": cosf, "sinf": sinf, "tri": tri, "iden": iden,
        })
    return in_maps


def run_on_cores(in_maps, trace=False, **kwargs):
    if "nc" not in _cache:
        _cache["nc"] = build_nc()
    return run_bass_kernel_spmd(
        _cache["nc"], in_maps, core_ids=list(range(N_CORES)), trace=trace,
        **kwargs)


def kernel(x, freqs_cos, freqs_sin, wq, wk, wv, wo):
    in_maps = make_in_maps(x, freqs_cos, freqs_sin, wq, wk, wv, wo)
    res = run_on_cores(in_maps)
    outs = [np.asarray(res.results[c]["out"], np.float32)
            for c in range(N_CORES)]
    full = np.empty((B, T, D), np.float32)
    for b in range(B):
        full[b] = outs[4 * b] + outs[4 * b + 1] + outs[4 * b + 2] + outs[4 * b + 3]
    return full
